# revision 69
# baseline (speedup 1.0000x reference)
"""Trainium2 Bass kernel for nn_BiMambaBlock — software-pipelined redesign.

Sharding: batch x sequence-halves -> 8 cores, 128-token halo, host
reverses time.  Key device-program structure:

  - Decay masks exp(S_t - S_s + lndt) are built ON THE PE as a rank-66
    matmul (indicator rows + S split into 16*K + R so the f32r operand
    rounding stays exact) and evicted through e^8*sigmoid(x-8), which
    equals exp(min(x, 8)) to 1e-3.  No DRAM mask broadcast, no DVE
    subtract/min.
  - The D*xh skip term rides the intra-chunk PSUM accumulation via
    identity matmuls; the state update (B(*)dtw masks + diag(elast)
    matmuls) accumulates into one packed [128,512] PSUM tile so the
    cross-tile state chain is just matmuls + one eviction.
  - The gated RMSNorm's per-token scale cancels in the downstream
    LayerNorm (rms_w folded into wout); only sum(yg^2) is kept and fed
    to the LN as a per-token eps correction (1e-5*ms/DI).
  - The tail (out_proj/LN/MLP) of tile i-1 is emitted as a queue of
    pieces interleaved into tile i's SSD stalls, so the PE stream
    always has dense matmul work; PSUM: pa/pc 2-buf heads, acc 2-buf
    1-bank accumulators.
  - Evictions are split across ACT/DVE/GPSIMD by time-window load;
    conv runs as PE diag-matmuls with block 8 (B/C) first.
"""

import numpy as np

# ---- dims ----
DM = 512          # d_model
DST = 64          # d_state
DI = 1024         # d_inner
NH = 16           # heads
HD = 64           # head dim
CD = 1152         # conv dim = DI + 2*DST
B, L = 4, 4096
EPS = 1e-5
HALO, SEG = 128, 2048
TOK = 256         # tokens per pipeline tile
NT = 1 + SEG // TOK   # 9 (halo half-tile + 8 full)
Q = 128           # SSD chunk
NCQ = TOK // Q    # chunks per tile

_BUILT = None
_FLAGS = None
DEBUG = False
REPEAT = 1


def _patch_concourse(tile_mod, bass_mod):
    """This container's walrus accepts a single sync-wait per instruction.
    Split extra waits onto NoOp / extra Drain instructions."""
    from concourse.vector_clock import ScopedClock
    import json

    def _drain_and_barrier(self, tick_clock, wait_clock):
        nc = self.nc
        drain_inst = nc.sync.drain()
        wait_clock.add_sem_waits(drain_inst.ins,
                                 ScopedClock({None: tick_clock.global_clock}))
        si = drain_inst.ins.sync_info
        waits = list(si.on_wait) if (si is not None and si.on_wait) else []
        if len(waits) > 1:
            si.on_wait = waits[:1]
            name2h = {h.name: h for h in self.sems.allocated().values()}
            for w in waits[1:]:
                d2 = nc.sync.drain()
                d2.wait_op(name2h[w.ant_name], w.wait_value, "sem-ge")
        nc.all_engine_barrier()
        popped = nc._tile_sem_poison_stack.pop()
        assert popped is self._sem_poison
        nc.clear_and_free_semaphores(list(self.sems.allocated().values()))
        nc.all_engine_barrier()

    tile_mod.TileContext._drain_and_barrier = _drain_and_barrier

    def _split_waits(m):
        n = 0
        for f in m.get("functions", []):
            for bb in f.get("blocks", []):
                out = []
                for ins in bb.get("instructions", []):
                    si = ins.get("sync_info")
                    waits = (si or {}).get("on_wait") or []
                    if len(waits) > 1:
                        for i, w in enumerate(waits[:-1]):
                            out.append({
                                "debug": ins.get("debug", 0),
                                "engine": ins["engine"],
                                "ins": [], "outs": [],
                                "name": f"{ins['name']}-ws{i}",
                                "opcode": "NoOp",
                                "sync_info": {"on_update": [], "on_wait": [w]},
                            })
                        si["on_wait"] = waits[-1:]
                        n += 1
                    out.append(ins)
                bb["instructions"] = out
        return n

    if not getattr(bass_mod.Bass, "_wait_split_patched", False):
        orig = bass_mod.Bass.to_json_bytes

        def to_json_bytes(self):
            raw = orig(self)
            m = json.loads(raw)
            if _split_waits(m):
                raw = json.dumps(m).encode()
            return raw

        bass_mod.Bass.to_json_bytes = to_json_bytes
        bass_mod.Bass._wait_split_patched = True


def _build(flags=("zb", "oneD")):
    global _BUILT, _FLAGS
    if _BUILT is not None and _FLAGS == flags:
        return _BUILT
    _FLAGS = flags
    zb = "zb" in flags      # conv_b, b1(+ln_b folded), b2 all zero
    oneD = "oneD" in flags  # D == 1
    import concourse.bass as bass
    import concourse.tile as tile
    from concourse import mybir
    from concourse.masks import make_identity
    from contextlib import ExitStack

    _patch_concourse(tile, bass)

    f32 = mybir.dt.float32
    f32r = mybir.dt.float32r
    bf16 = mybir.dt.bfloat16
    AL = mybir.AluOpType
    AF = mybir.ActivationFunctionType

    nc = bass.Bass()

    # ---- DRAM I/O (per-core) ----
    xT = nc.dram_tensor("xT", (4, 128, HALO + SEG), f32r, kind="ExternalInput")
    wz = nc.dram_tensor("wz", (4, 128, DI), f32r, kind="ExternalInput")
    wxbc = nc.dram_tensor("wxbc", (4, 128, CD), f32r, kind="ExternalInput")
    wdt = nc.dram_tensor("wdt", (4, 128, NH), f32r, kind="ExternalInput")
    wout = nc.dram_tensor("wout", (8, 128, DM), bf16, kind="ExternalInput")
    w1 = nc.dram_tensor("w1", (4, 128, DI), bf16, kind="ExternalInput")
    w2 = nc.dram_tensor("w2", (8, 128, DM), bf16, kind="ExternalInput")
    convw = nc.dram_tensor("convw", (128, 9, 4), f32, kind="ExternalInput")
    convb = nc.dram_tensor("convb", (128, 9, 1), f32, kind="ExternalInput")
    dtb = nc.dram_tensor("dtb", (NH, 1), f32, kind="ExternalInput")
    Ah = nc.dram_tensor("Ah", (NH, 1), f32, kind="ExternalInput")
    Drep = nc.dram_tensor("Drep", (1, DI), bf16, kind="ExternalInput")
    b1r = nc.dram_tensor("b1r", (1, 8, 128), f32, kind="ExternalInput")
    b2r = nc.dram_tensor("b2r", (1, 4, 128), f32, kind="ExternalInput")
    cbr = nc.dram_tensor("cbr", (1, 8, 128), f32, kind="ExternalInput")
    triu = nc.dram_tensor("triu", (128, 128), f32, kind="ExternalInput")
    indr = nc.dram_tensor("indr", (66, NH * Q + NH), f32r,
                          kind="ExternalInput")
    indm = nc.dram_tensor("indm", (66, 2 * NH), f32r, kind="ExternalInput")
    ltc2 = nc.dram_tensor("ltc2", (2, 128), f32r, kind="ExternalInput")
    outT = nc.dram_tensor("outT", (4, 128, SEG), f32, kind="ExternalOutput")
    dbg = {}
    if DEBUG:
        for nm, shp, dt_ in [
                ("d_ub", (128, NH, Q), bf16), ("d_dtw", (128, NH), bf16),
                ("d_es", (128, NH), bf16), ("d_cbt", (128, 128), bf16),
                ("d_state", (128, 512), bf16), ("d_ynt", (128, DI), bf16),
                ("d_xh", (128, DI), bf16), ("d_z", (128, DI), bf16),
                ("d_ym", (128, 4, TOK), bf16), ("d_ln", (128, 4, TOK), bf16),
                ("d_sfm", (NH, Q), f32), ("d_lt", (66, Q), f32),
                ("d_rh", (66, NH * Q + NH), f32),
                ("d_conv", (128, 9, TOK), bf16)]:
            dbg[nm] = nc.dram_tensor(nm, shp, dt_, kind="ExternalOutput")

    with tile.TileContext(nc) as tc, ExitStack() as ctx:
        wp = ctx.enter_context(tc.tile_pool(name="wp", bufs=1))
        xp = ctx.enter_context(tc.tile_pool(name="xp", bufs=2))
        cq = ctx.enter_context(tc.tile_pool(name="cq", bufs=2))   # per-tile mids
        sp = ctx.enter_context(tc.tile_pool(name="sp", bufs=2))   # small per-chunk
        mk = ctx.enter_context(tc.tile_pool(name="mk", bufs=2))   # masks
        yp = ctx.enter_context(tc.tile_pool(name="yp", bufs=1))   # big per-chunk
        zp = ctx.enter_context(tc.tile_pool(name="zp", bufs=2))   # xh, z
        st = ctx.enter_context(tc.tile_pool(name="st", bufs=2))   # state & stash
        op = ctx.enter_context(tc.tile_pool(name="op", bufs=2))   # out stage
        dp = ctx.enter_context(tc.tile_pool(name="dp", bufs=2, space="DRAM"))
        # PSUM rings: head (in_proj/conv/dt), chunk (z/transposes/CBt),
        # accumulators (ypsA/ypsB/yn-transposes/pdh, one serial slot),
        # tail (LN stats/out_proj/MLP).  2+2+2+2 banks.
        pa = ctx.enter_context(tc.tile_pool(name="pa", bufs=2, space="PSUM"))
        pc = ctx.enter_context(tc.tile_pool(name="pc", bufs=2, space="PSUM"))
        acc = ctx.enter_context(tc.tile_pool(name="acc", bufs=2, space="PSUM"))
        pt = ctx.enter_context(tc.tile_pool(name="pt", bufs=2, space="PSUM"))

        # ---- load weights / constants ----
        def ld(name, dram, shape, dt=f32):
            t = wp.tile(list(shape), dt, tag=name)
            nc.sync.dma_start(out=t[:], in_=dram[:])
            return t

        t_wz = [ld(f"wz{k}", wz[k], (128, DI), f32r) for k in range(4)]
        t_wxbc = [ld(f"wxbc{k}", wxbc[k], (128, CD), f32r) for k in range(4)]
        t_wdt = [ld(f"wdt{k}", wdt[k], (128, NH), f32r) for k in range(4)]
        t_wout = [ld(f"wout{k}", wout[k], (128, DM), bf16) for k in range(8)]
        t_w1 = [ld(f"w1{k}", w1[k], (128, DI), bf16) for k in range(4)]
        t_w2 = [ld(f"w2{k}", w2[k], (128, DM), bf16) for k in range(8)]
        t_convw = ld("convw", convw, (128, 9, 4))
        t_convb = None if zb else ld("convb", convb, (128, 9, 1))
        t_dtb = ld("dtb", dtb, (NH, 1))
        t_A = ld("Ah", Ah, (NH, 1))
        t_b1r = None if zb else ld("b1r", b1r, (1, 8, 128))
        t_b2r = None if zb else ld("b2r", b2r, (1, 4, 128))
        t_cbr = None if zb else ld("cbr", cbr, (1, 8, 128))
        t_triu = ld("triu", triu, (128, 128))
        t_Dbc = None
        if not oneD:
            t_Dbc = wp.tile([128, DI], bf16, tag="Dbc")
            nc.sync.dma_start(out=t_Dbc[:], in_=Drep[:].to_broadcast((128, DI)))
        ident = wp.tile([128, 128], f32, tag="ident")
        make_identity(nc, ident[:])
        ident_bf = wp.tile([128, 128], bf16, tag="ident_bf")
        nc.vector.tensor_copy(ident_bf[:], ident[:])
        E8 = float(np.exp(8.0))
        ident64_e8 = wp.tile([128, DST], bf16, tag="ident64_e8")
        nc.vector.tensor_scalar(ident64_e8[0:DST, :], ident[0:DST, 0:DST],
                                E8, None, op0=AL.mult)
        nc.vector.tensor_scalar(ident64_e8[DST:128, :], ident[0:DST, 0:DST],
                                E8, None, op0=AL.mult)
        onerow_bf = wp.tile([1, 128], bf16, tag="onerow_bf")
        nc.vector.memset(onerow_bf[:], 1.0)
        # decay-diff matmul rhs: row0 = S-row (dynamic), rows 1..16 = head
        # indicators, cols 2048:2064 = S_last block (ind rows zero there)
        rh_t = []
        for par in range(2):
            t = wp.tile([66, NH * Q + NH], f32r, tag=f"rh{par}",
                        name=f"rh{par}")
            nc.sync.dma_start(out=t[:], in_=indr[:])
            rh_t.append(t)
        rhm = wp.tile([66, 2 * NH], f32r, tag="rhm")
        nc.sync.dma_start(out=rhm[:], in_=indm[:])
        t_ltc2 = wp.tile([2, 128], f32r, tag="ltc2")
        nc.sync.dma_start(out=t_ltc2[:], in_=ltc2[:])
        # conv diag matrices: convd[:, mt, k, :] = diag(convw[:, mt, k]) bf16
        convd = wp.tile([128, 9, 4, 128], bf16, tag="convd")
        for mt in range(9):
            for k in range(4):
                nc.vector.tensor_scalar(convd[:, mt, k, :], ident_bf[:],
                                        t_convw[:, mt, k:k + 1], None,
                                        op0=AL.mult)
        zeros16 = wp.tile([NH, 1], f32, tag="zeros16")
        nc.vector.memset(zeros16[:], 0.0)
        ones16 = wp.tile([NH, 1], f32, tag="ones16")
        nc.vector.memset(ones16[:], 1.0)
        epsc = wp.tile([128, 1], f32, tag="epsc")
        nc.vector.memset(epsc[:], EPS)
        neg8 = wp.tile([128, 1], f32, tag="neg8")
        nc.vector.memset(neg8[:], -8.0)
        z66 = wp.tile([66, 128], f32, tag="z66")
        nc.vector.memset(z66[:], 0.0)
        ones32 = wp.tile([128, 1], f32, tag="ones32")
        nc.vector.memset(ones32[:], 1.0)
        ones_bf = wp.tile([128, 1], bf16, tag="ones_bf")
        nc.vector.tensor_copy(ones_bf[:], ones32[:])
        onerow = None
        if not zb:
            onerow = wp.tile([1, 256], f32, tag="onerow")
            nc.vector.memset(onerow[:], 1.0)

        # persistent state, packed [ (h//8)*64+n , (h%8)*64+d ]
        state = st.tile([128, 512], bf16, tag="state")
        nc.vector.memset(state[:], 0.0)
        stash = st.tile([128, 9, 3], bf16, tag="stash")
        nc.vector.memset(stash[:], 0.0)

        for _rep in range(REPEAT):
            _ = _rep
            tailq = []

            def fill(n=1):
                for _ in range(min(n, len(tailq))):
                    tailq.pop(0)()

            def _emit_tail(it_, t0_, yn_, ssr_):
                tl = {}

                def p_out(mp_):
                    def f():
                        if "ym" not in tl:
                            tl["ym"] = op.tile([128, 4, TOK], bf16,
                                               tag="ym_fm", name="ym_fm")
                        ym_fm = tl["ym"]
                        ps = pt.tile([128, 512], f32, tag="pt", name="ps")
                        for j in range(2):
                            mt = 2 * mp_ + j
                            for k in range(8):
                                nc.tensor.matmul(
                                    ps[:, j * 256:(j + 1) * 256],
                                    t_wout[k][:, mt * 128:(mt + 1) * 128],
                                    yn_[:, k, :], start=(k == 0),
                                    stop=(k == 7))
                        nc.scalar.copy(
                            ym_fm[:, 2 * mp_:2 * mp_ + 2, :]
                                .rearrange("p a b -> p (a b)"), ps[:])
                        if DEBUG and it_ == 1 and mp_ == 1:
                            nc.sync.dma_start(out=dbg["d_ym"][:], in_=ym_fm[:])
                    return f

                def p_lnstats():
                    ym_fm = tl["ym"]
                    sq4 = op.tile([128, 4, TOK], bf16, tag="sq4", name="sq4")
                    nc.vector.tensor_tensor(out=sq4[:], in0=ym_fm[:],
                                            in1=ym_fm[:], op=AL.mult)
                    pmu = pt.tile([1, TOK], f32, tag="pt", name="pmu")
                    for k in range(4):
                        nc.tensor.matmul(pmu[:], ones_bf[:], ym_fm[:, k, :],
                                         start=(k == 0), stop=(k == 3))
                    pv = pt.tile([1, TOK], f32, tag="pt", name="pv")
                    for k in range(4):
                        nc.tensor.matmul(pv[:], ones_bf[:], sq4[:, k, :],
                                         start=(k == 0), stop=(k == 3))
                    murs = sp.tile([1, 2, TOK], bf16, tag="murs", name="murs")
                    nc.scalar.activation(murs[:, 0, :], pmu[:], AF.Copy,
                                         scale=1.0 / DM)
                    ex2_row = sp.tile([1, TOK], f32, tag="ex2_row",
                                      name="ex2_row")
                    nc.scalar.activation(ex2_row[:], pv[:], AF.Copy,
                                         scale=1.0 / DM)
                    var_row = sp.tile([1, TOK], f32, tag="var_row",
                                      name="var_row")
                    nc.vector.tensor_tensor(out=var_row[:], in0=murs[:, 0, :],
                                            in1=murs[:, 0, :], op=AL.mult)
                    nc.vector.tensor_tensor(out=var_row[:], in0=ex2_row[:],
                                            in1=var_row[:], op=AL.subtract)
                    nc.vector.scalar_tensor_tensor(
                        out=var_row[:], in0=ssr_[:], scalar=EPS / DI,
                        in1=var_row[:], op0=AL.mult, op1=AL.add)
                    nc.scalar.activation(var_row[:], var_row[:], AF.Ln,
                                         bias=0.0, scale=1.0)
                    nc.scalar.activation(murs[:, 1, :], var_row[:], AF.Exp,
                                         scale=-0.5)
                    pbc = pc.tile([128, 2, TOK], f32, tag="pc", name="pbc")
                    nc.tensor.matmul(pbc[:].rearrange("p a b -> p (a b)"),
                                     onerow_bf[:],
                                     murs[:].rearrange("p a b -> p (a b)"),
                                     start=True, stop=True)
                    mrbc = sp.tile([128, 2, TOK], bf16, tag="mrbc",
                                   name="mrbc")
                    nc.scalar.copy(mrbc[:].rearrange("p a b -> p (a b)"),
                                   pbc[:].rearrange("p a b -> p (a b)"))
                    ln_fm = op.tile([128, 4, TOK], bf16, tag="sq4",
                                    name="ln_fm")
                    nc.vector.tensor_tensor(
                        out=ln_fm[:], in0=ym_fm[:],
                        in1=mrbc[:, 0:1, :].to_broadcast((128, 4, TOK)),
                        op=AL.subtract)
                    nc.vector.tensor_tensor(
                        out=ln_fm[:], in0=ln_fm[:],
                        in1=mrbc[:, 1:2, :].to_broadcast((128, 4, TOK)),
                        op=AL.mult)
                    if DEBUG and it_ == 1:
                        nc.sync.dma_start(out=dbg["d_ln"][:], in_=ln_fm[:])
                    tl["ln"] = ln_fm

                def p_w1(g):
                    def f():
                        if "h" not in tl:
                            tl["h"] = op.tile([128, 8, TOK], bf16,
                                              tag="h_fm", name="h_fm")
                        h_fm = tl["h"]
                        ln_fm = tl["ln"]
                        for mp_ in (2 * g, 2 * g + 1):
                            ps = pt.tile([128, 512], f32, tag="pt", name="ps")
                            for j in range(2):
                                mt = 2 * mp_ + j
                                reg = ps[:, j * 256:(j + 1) * 256]
                                for k in range(4):
                                    nc.tensor.matmul(
                                        reg,
                                        t_w1[k][:, mt * 128:(mt + 1) * 128],
                                        ln_fm[:, k, :], start=(k == 0),
                                        stop=(zb and k == 3))
                                if not zb:
                                    nc.tensor.matmul(reg, t_b1r[0:1, mt, :],
                                                     onerow[:], start=False,
                                                     stop=True)
                            nc.scalar.activation(
                                h_fm[:, 2 * mp_:2 * mp_ + 2, :]
                                    .rearrange("p a b -> p (a b)"),
                                ps[:], AF.Silu)
                    return f

                def p_w2(g):
                    def f():
                        h_fm = tl["h"]
                        for mt in (2 * g, 2 * g + 1):
                            ps = pt.tile([128, TOK], f32, tag="pt", name="ps")
                            for k in range(8):
                                nc.tensor.matmul(
                                    ps[:],
                                    t_w2[k][:, mt * 128:(mt + 1) * 128],
                                    h_fm[:, k, :], start=(k == 0),
                                    stop=(zb and k == 7))
                            if not zb:
                                nc.tensor.matmul(ps[:], t_b2r[0:1, mt, :],
                                                 onerow[:], start=False,
                                                 stop=True)
                            ot = op.tile([128, TOK], f32, tag="ot", name="ot")
                            nc.scalar.copy(ot[:], ps[:])
                            nc.sync.dma_start(
                                out=outT[mt][:, t0_ - HALO:t0_ - HALO + TOK],
                                in_=ot[:])
                    return f

                tailq.extend([p_out(0), p_out(1), p_lnstats,
                              p_w1(0), p_w1(1), p_w2(0), p_w2(1)])

            for it in range(NT):
                tt = HALO if it == 0 else TOK
                nct = 1 if it == 0 else NCQ
                t0 = 0 if it == 0 else HALO + (it - 1) * TOK
                # ---- x tile (feature-major), one DMA ----
                x_fm = xp.tile([128, 4, tt], f32r, tag="x_fm",
                               padded_shape=[128, 4, TOK])
                for k in range(4):
                    nc.sync.dma_start(out=x_fm[:, k, :],
                                      in_=xT[k][:, t0:t0 + tt])

                # ---- dt path first: its scan + DMA latency feeds the
                # decay matmuls, so start it before the big GEMMs ----
                psd = pa.tile([NH, tt], f32, tag="pa",
                              padded_shape=[NH, TOK])
                for k in range(4):
                    nc.tensor.matmul(psd[:], t_wdt[k][:], x_fm[:, k, :],
                                     start=(k == 0), stop=(k == 3))
                dt_fm = cq.tile([NH, tt], f32, tag="dt_fm",
                                padded_shape=[NH, TOK])
                nc.scalar.activation(dt_fm[:], psd[:], AF.Exp, bias=t_dtb[:],
                                     scale=1.0)
                nc.scalar.activation(dt_fm[:], dt_fm[:], AF.Ln, bias=ones16[:],
                                     scale=1.0)
                dtA_fm = cq.tile([NH, tt], f32, tag="dtA_fm",
                                 padded_shape=[NH, TOK])
                nc.vector.tensor_scalar_mul(dtA_fm[:], dt_fm[:], t_A[:])
                lndt_fm = cq.tile([NH, tt], f32, tag="lndt_fm",
                                  padded_shape=[NH, TOK])
                nc.scalar.activation(lndt_fm[:], dt_fm[:], AF.Ln)
                s_fm = []
                for c in range(nct):
                    sf = sp.tile([NH, Q], f32, tag=f"s_fm{c}")
                    nc.vector.tensor_tensor_scan(
                        sf[:], dtA_fm[:, c * 128:(c + 1) * 128],
                        zeros16[:].to_broadcast((NH, Q)),
                        0.0, op0=AL.add, op1=AL.add)
                    s_fm.append(sf)
                # decay operand prep: f32r rounds operands to ~12 mantissa
                # bits, so the raw cumsum (|S| up to ~6k) loses the ~1e-3
                # absolute precision the decay mask needs.  Split
                # S = 16*K + R (K exactly representable, R in (-16,16)) and
                # carry K and R through separate contraction rows:
                #   lt rows 0..15  = lndt - R   (paired with +1 ind)
                #   lt rows 32..47 = K          (paired with -16 ind)
                #   lt row 64 = 1 (pairs R-row), row 65 = 16 (K-row)
                lt_t = []
                for c in range(nct):
                    Kb = sp.tile([NH, Q], bf16, tag=f"Kb{c}")
                    nc.vector.tensor_scalar(Kb[:], s_fm[c][:], 1.0 / 16.0,
                                            None, op0=AL.mult)
                    Km = sp.tile([NH, Q], f32, tag=f"Km{c}")
                    nc.vector.tensor_copy(Km[:], Kb[:])
                    Rm = sp.tile([NH, Q], f32, tag=f"Rm{c}")
                    nc.vector.scalar_tensor_tensor(
                        out=Rm[:], in0=Km[:], scalar=-16.0, in1=s_fm[c][:],
                        op0=AL.mult, op1=AL.add)
                    lt = sp.tile([66, Q], f32r, tag=f"lt{c}")
                    nc.vector.tensor_copy(lt[:], z66[:])
                    nc.vector.tensor_copy(lt[64:66, :], t_ltc2[:])
                    nc.vector.tensor_tensor(
                        out=lt[0:NH, :], in0=lndt_fm[:, c * Q:(c + 1) * Q],
                        in1=Rm[:], op=AL.subtract)
                    nc.vector.tensor_copy(lt[32:32 + NH, :], Km[:])
                    lt_t.append(lt)
                    if it > 0:
                        rh = rh_t[c]
                        nc.sync.dma_start(out=rh[64:65, :NH * Q],
                                          in_=Rm[:].bitcast(f32r))
                        nc.sync.dma_start(out=rh[64:65, NH * Q:],
                                          in_=Rm[:, Q - 1:Q].bitcast(f32r))
                        nc.sync.dma_start(out=rh[65:66, :NH * Q],
                                          in_=Km[:].bitcast(f32r))
                        nc.sync.dma_start(out=rh[65:66, NH * Q:],
                                          in_=Km[:, Q - 1:Q].bitcast(f32r))
                    else:
                        nc.sync.dma_start(out=rhm[64:65, 0:NH],
                                          in_=Rm[:, Q - 1:Q].bitcast(f32r))
                        nc.sync.dma_start(out=rhm[64:65, NH:2 * NH],
                                          in_=Rm[:, Q - 1:Q].bitcast(f32r))
                        nc.sync.dma_start(out=rhm[65:66, 0:NH],
                                          in_=Km[:, Q - 1:Q].bitcast(f32r))
                        nc.sync.dma_start(out=rhm[65:66, NH:2 * NH],
                                          in_=Km[:, Q - 1:Q].bitcast(f32r))

                # ---- in_proj: paired blocks -> PSUM -> bf16 (conv halo) ----
                xbc_ext = cq.tile([128, 9, 3 + tt], bf16, tag="xbc_ext",
                                  padded_shape=[128, 9, 3 + TOK])
                nc.vector.tensor_copy(xbc_ext[:, :, 0:3], stash[:])
                for mts in ([8], [0, 1], [2, 3], [4, 5], [6, 7]):
                    w = tt * len(mts)
                    ps = pa.tile([128, w], f32, tag="pa",
                                 padded_shape=[128, 512])
                    for j, mt in enumerate(mts):
                        for k in range(4):
                            nc.tensor.matmul(
                                ps[:, j * tt:(j + 1) * tt],
                                t_wxbc[k][:, mt * 128:(mt + 1) * 128],
                                x_fm[:, k, :], start=(k == 0), stop=(k == 3))
                    nc.vector.tensor_copy(
                        xbc_ext[:, mts[0]:mts[0] + len(mts), 3:3 + tt], ps[:])
                stash_new = st.tile([128, 9, 3], bf16, tag="stash")
                nc.vector.tensor_copy(stash_new[:], xbc_ext[:, :, tt:tt + 3])
                stash = stash_new

                # ---- conv: paired diag matmuls + SiLU evict (block 8 first
                # so the B/C features are ready early) ----
                conv_all = cq.tile([128, 9, tt], bf16, tag="conv_all",
                                   padded_shape=[128, 9, TOK])
                for mts in ([8], [0, 1], [2, 3], [4, 5], [6, 7]):
                    w = tt * len(mts)
                    pcv = pa.tile([128, w], f32, tag="pa",
                                  padded_shape=[128, 512])
                    for j, mt in enumerate(mts):
                        reg = pcv[:, j * tt:(j + 1) * tt]
                        for k in range(4):
                            nc.tensor.matmul(reg, convd[:, mt, k, :],
                                             xbc_ext[:, mt, k:k + tt],
                                             start=(k == 0),
                                             stop=((zb or mt == 8) and k == 3))
                        if mt != 8 and not zb:
                            nc.tensor.matmul(reg, t_cbr[0:1, mt, :],
                                             onerow[:, 0:tt],
                                             start=False, stop=True)
                    if mts == [8]:
                        nc.scalar.activation(conv_all[:, 8, :], pcv[:], AF.Silu,
                                             bias=(0.0 if zb else t_convb[:, 8, :]))
                    elif tt == TOK:
                        nc.scalar.activation(
                            conv_all[:, mts[0]:mts[0] + 2, :]
                                .rearrange("p a b -> p (a b)"),
                            pcv[:], AF.Silu)
                    else:
                        for j, mt in enumerate(mts):
                            nc.scalar.activation(
                                conv_all[:, mt, :],
                                pcv[:, j * tt:(j + 1) * tt], AF.Silu)
                conv_s = [conv_all[:, mt, :] for mt in range(9)]
                # ---- B/C prep (conv block 8 first => early) ----
                C_ts, B_tms = [], []
                for c in range(nct):
                    csl = slice(c * 128, (c + 1) * 128)
                    C_t = sp.tile([128, Q], bf16, tag=f"C_t{c}", name="C_t")
                    nc.sync.dma_start(out=C_t[0:DST, :],
                                      in_=conv_s[8][DST:128, csl])
                    nc.sync.dma_start(out=C_t[DST:128, :],
                                      in_=conv_s[8][DST:128, csl])
                    pbt = pc.tile([128, DST], bf16, tag="pc", name="pbt")
                    nc.tensor.transpose(pbt[:], conv_s[8][0:DST, csl],
                                        ident_bf[0:DST, 0:DST])
                    B_tm = sp.tile([128, DST], bf16, tag=f"B_tm{c}",
                                   name="B_tm")
                    nc.vector.tensor_copy(B_tm[:], pbt[:])
                    C_ts.append(C_t)
                    B_tms.append(B_tm)

                # ---- z (token-major, silu) ----
                silu_z = [None, None]
                if it > 0:
                    for c in range(NCQ):
                        zt = zp.tile([128, DI], bf16, tag=f"silu_z{c}")
                        for h2 in range(2):
                            ps = pc.tile([128, 512], f32, tag="pc")
                            for k in range(4):
                                nc.tensor.matmul(
                                    ps[:], x_fm[:, k, c * 128:(c + 1) * 128],
                                    t_wz[k][:, h2 * 512:(h2 + 1) * 512],
                                    start=(k == 0), stop=(k == 3))
                            nc.scalar.activation(zt[:, h2 * 512:(h2 + 1) * 512],
                                                 ps[:], AF.Silu)
                        silu_z[c] = zt

                # ---- xh token-major (transpose + DVE evict) ----
                xh_tm = [None, None]
                for c in range(nct):
                    csl = slice(c * 128, (c + 1) * 128)
                    xt_ = zp.tile([128, DI], bf16, tag=f"xh_tm{c}")
                    ptx = pc.tile([128, 8, 128], bf16, tag="pc")
                    for mt in range(8):
                        nc.tensor.transpose(ptx[:, mt, :], conv_s[mt][:, csl],
                                            ident_bf[:])
                    nc.vector.tensor_copy(xt_[:], ptx[:].rearrange("p a b -> p (a b)"))
                    xh_tm[c] = xt_

                # ---- decay masks (sigmoid evicts land right before their
                # consumption in the chunk loop) ----
                E_tmb = None
                if it > 0:
                    pstb = pc.tile([128, 2, NH], f32, tag="pc")
                    for c in range(NCQ):
                        nc.tensor.transpose(pstb[:, c, :], s_fm[c][:],
                                            ident[0:NH, 0:NH])
                    E_tmb = sp.tile([128, 2, NH], bf16, tag="E_tmb")
                    nc.scalar.activation(E_tmb[:], pstb[:], AF.Exp)
                ubf = [None, None]
                dtw = [None, None]
                es = [None, None]
                mpds, des = [], []
                for c in range(nct):
                    lt = lt_t[c]
                    if it > 0:
                        rh = rh_t[c]
                        ub = mk.tile([128, NH, Q], bf16, tag="ubf")
                        ubfl = ub[:].rearrange("p h q -> p (h q)")
                        for j in range(4):
                            pd = pc.tile([128, 512], f32, tag="pc")
                            nc.tensor.matmul(pd[:], lt[:],
                                             rh[:, j * 512:(j + 1) * 512],
                                             start=True, stop=True)
                            nc.scalar.activation(
                                ubfl[:, j * 512:(j + 1) * 512], pd[:],
                                AF.Sigmoid, bias=neg8[:], scale=1.0)
                        pe_t = pc.tile([128, NH], f32, tag="pc")
                        nc.tensor.matmul(pe_t[:], lt[:],
                                         rh[:, NH * Q:],
                                         start=True, stop=True)
                        ubf[c] = ub
                    else:
                        pm = pc.tile([128, 2 * NH], f32, tag="pc")
                        nc.tensor.matmul(pm[:], lt[:],
                                         rhm[:],
                                         start=True, stop=True)
                        um = sp.tile([128, 2 * NH], bf16, tag=f"um{c}")
                        nc.scalar.activation(um[:], pm[:], AF.Sigmoid,
                                             bias=neg8[:], scale=1.0)
                        pe_t = None
                    dw = sp.tile([128, NH], bf16, tag=f"dtw{c}")
                    esb = sp.tile([128, NH], bf16, tag=f"es{c}")
                    if it > 0:
                        nc.vector.tensor_scalar(dw[:], ubf[c][:, :, Q - 1],
                                                E8, None, op0=AL.mult)
                        nc.scalar.activation(esb[:], pe_t[:],
                                             AF.Sigmoid, bias=neg8[:],
                                             scale=1.0)
                    else:
                        nc.vector.tensor_scalar(dw[:], um[:, 0:NH],
                                                E8, None, op0=AL.mult)
                        nc.vector.tensor_copy(esb[:], um[:, NH:2 * NH])
                    if DEBUG and it == 1 and c == 0:
                        nc.sync.dma_start(out=dbg["d_ub"][:], in_=ubf[c][:])
                        nc.sync.dma_start(out=dbg["d_dtw"][:], in_=dw[:])
                        nc.sync.dma_start(out=dbg["d_es"][:], in_=esb[:])
                        nc.sync.dma_start(out=dbg["d_sfm"][:], in_=s_fm[c][:])
                        nc.sync.dma_start(out=dbg["d_lt"][:],
                                          in_=lt[:].bitcast(f32))
                        nc.sync.dma_start(out=dbg["d_rh"][:],
                                          in_=rh_t[0][:].bitcast(f32))
                    dtw[c] = dw
                    es[c] = esb
                    mpd = yp.tile([128, NH, DST], bf16, tag=f"mpd{c}",
                                  name="mpd")
                    eng_ = nc.vector if c == 0 else nc.gpsimd
                    eng_.tensor_tensor(
                        out=mpd[:],
                        in0=B_tms[c][:].rearrange("p (o n) -> p o n", o=1)
                            .to_broadcast((128, NH, DST)),
                        in1=dw[:].rearrange("p (h o) -> p h o", o=1)
                            .to_broadcast((128, NH, DST)),
                        op=AL.mult)
                    de = yp.tile([128, NH, DST], bf16, tag=f"de{c}", name="de")
                    nc.gpsimd.tensor_tensor(
                        out=de[:],
                        in0=ident64_e8[:].rearrange("p (o n) -> p o n", o=1)
                            .to_broadcast((128, NH, DST)),
                        in1=esb[:].rearrange("p (h o) -> p h o", o=1)
                            .to_broadcast((128, NH, DST)),
                        op=AL.mult)
                    mpds.append(mpd)
                    des.append(de)

                # ---- per 128-chunk SSD ----
                yn_fm = None
                ssr = None
                if it > 0:
                    yn_fm = op.tile([128, 8, TOK], bf16, tag="yn_fm")
                    ssr = sp.tile([1, TOK], f32, tag="ssr", name="ssr")
                for c in range(nct):
                    csl = slice(c * 128, (c + 1) * 128)
                    B_fm = conv_s[8][0:DST, csl]
                    C_t = C_ts[c]
                    mpd = mpds[c]
                    de = des[c]
                    # ---- state update first (needs only mpd/de + old
                    # state, so state(c+1) is ready as early as possible):
                    #   pdh[n,(h,d)] = sum_s B[s,n]*dtw[s,h]*xh[s,(h,d)]
                    #               + elast_h * state[n,(h,d)]   (packed)
                    pdh = acc.tile([128, 512], f32, tag="acc", name="pdh")
                    for h in range(NH):
                        hb, hr = divmod(h, 8)
                        reg = pdh[hb * 64:(hb + 1) * 64,
                                  hr * HD:(hr + 1) * HD]
                        st_sl = state[hb * 64:(hb + 1) * 64,
                                      hr * HD:(hr + 1) * HD]
                        nc.tensor.matmul(reg, mpd[:, h, :],
                                         xh_tm[c][:, h * HD:(h + 1) * HD],
                                         start=True, stop=False)
                        nc.tensor.matmul(
                            reg, de[hb * 64:(hb + 1) * 64, h, :], st_sl,
                            start=False, stop=True)
                    state_new = st.tile([128, 512], bf16, tag="state")
                    nc.scalar.copy(state_new[:], pdh[:])
                    if it > 0:
                        # CBt (shared across heads), upper-tri masked
                        pcb = pc.tile([128, 128], f32, tag="pc")
                        nc.tensor.matmul(pcb[:], B_fm, C_t[0:DST, :],
                                         start=True, stop=True)
                        CBt = sp.tile([128, 128], bf16, tag="CBt")
                        nc.vector.tensor_tensor(out=CBt[:], in0=pcb[:],
                                                in1=t_triu[:], op=AL.mult)
                        if DEBUG and it == 1 and c == 0:
                            nc.sync.dma_start(out=dbg["d_cbt"][:], in_=CBt[:])
                        mth = ubf[c]
                        nc.vector.tensor_tensor(
                            out=mth[:], in0=ubf[c][:],
                            in1=CBt[:].rearrange("p (o q) -> p o q", o=1)
                                .to_broadcast((128, NH, Q)),
                            op=AL.mult)

                        if oneD:
                            Dxh = xh_tm[c]
                        else:
                            Dxh = yp.tile([128, DI], bf16, tag="Dxh")
                            nc.gpsimd.tensor_tensor(out=Dxh[:], in0=xh_tm[c][:],
                                                    in1=t_Dbc[:], op=AL.mult)
                        Yt = yp.tile([128, NH, HD], bf16, tag="Yt",
                                     bufs=2)
                        Ytf = Yt[:].rearrange("p h d -> p (h d)")
                        for hb in range(2):
                            # inter-chunk: C @ state-half, scaled by E
                            yB = acc.tile([128, 512], f32, tag="acc",
                                          name="yB")
                            nc.tensor.matmul(yB[:],
                                             C_t[hb * 64:(hb + 1) * 64, :],
                                             state[hb * 64:(hb + 1) * 64, :],
                                             start=True, stop=True)
                            nc.vector.tensor_tensor(
                                out=Yt[:, hb * 8:(hb + 1) * 8, :],
                                in0=yB[:].rearrange("p (h d) -> p h d", h=8),
                                in1=E_tmb[:, c:c + 1, hb * 8:(hb + 1) * 8]
                                    .rearrange("p o h -> p h o")
                                    .to_broadcast((128, 8, HD)),
                                op=AL.mult)
                            if hb == 0:
                                fill(1)
                            # intra-chunk masked matmuls + D*xh skip term
                            yA = acc.tile([128, 512], f32, tag="acc",
                                          name="yA")
                            for hr in range(8):
                                h = hb * 8 + hr
                                reg = yA[:, hr * HD:(hr + 1) * HD]
                                nc.tensor.matmul(
                                    reg, mth[:, h, :],
                                    xh_tm[c][:, h * HD:(h + 1) * HD],
                                    start=True, stop=False)
                                nc.tensor.matmul(
                                    reg, ident_bf[:],
                                    Dxh[:, h * HD:(h + 1) * HD],
                                    start=False, stop=True)
                            nc.vector.tensor_tensor(
                                out=Ytf[:, hb * 512:(hb + 1) * 512],
                                in0=Ytf[:, hb * 512:(hb + 1) * 512],
                                in1=yA[:], op=AL.add)
                        nc.vector.tensor_tensor(out=Ytf, in0=Ytf,
                                                in1=silu_z[c][:], op=AL.mult)
                    state = state_new
                    if it > 0:
                        if DEBUG and it == 1 and c == 0:
                            nc.sync.dma_start(out=dbg["d_ynt"][:], in_=Ytf)
                        fill(2)
                        # transpose y-gated -> feature-major (batched evict)
                        ptn = pc.tile([128, 8, 128], bf16, tag="pc",
                                      name="ptn")
                        for mt in range(8):
                            nc.tensor.transpose(ptn[:, mt, :],
                                                Ytf[:, mt * 128:(mt + 1) * 128],
                                                ident_bf[:])
                        nc.vector.tensor_copy(yn_fm[:, :, csl], ptn[:])
                        # per-token sum(yg^2): the RMS scale itself cancels
                        # in the downstream LayerNorm, but its eps does not;
                        # ship ms to the LN as a per-token eps correction.
                        # Emitted after the yn evict so it stays off the
                        # chunk's forward chain.
                        sqd = yp.tile([128, DI], bf16, tag="sqd")
                        ss = sp.tile([128, 1], f32, tag=f"ss{c}")
                        nc.vector.scalar_tensor_tensor(
                            out=sqd[:], in0=Ytf, scalar=1.0, in1=Ytf,
                            op0=AL.mult, op1=AL.mult, accum_out=ss[:])
                        pss = pc.tile([1, Q], f32, tag="pc", name="pss")
                        nc.tensor.transpose(pss[:], ss[:], ident[:])
                        nc.scalar.copy(ssr[:, c * Q:(c + 1) * Q], pss[:])

                if it > 0:
                    _emit_tail(it, t0, yn_fm, ssr)
                while len(tailq) > 7:
                    fill(1)
            if _rep == REPEAT - 1:
                while tailq:
                    fill(1)


    _BUILT = nc
    return nc


def _make_indr():
    ind = np.zeros((66, NH * 128 + NH), np.float32)
    for h in range(NH):
        ind[h, h * 128:(h + 1) * 128] = 1.0
        ind[32 + h, h * 128:(h + 1) * 128] = -16.0
    return ind


def _make_indm():
    ind = np.zeros((66, 2 * NH), np.float32)
    for h in range(NH):
        ind[h, h] = 1.0
        ind[32 + h, h] = -16.0
    return ind


def _make_ltc2():
    c = np.empty((2, 128), np.float32)
    c[0] = 1.0
    c[1] = 16.0
    return c


def _host_prep(inputs):
    import ml_dtypes
    bf = ml_dtypes.bfloat16
    x = np.asarray(inputs["x"], np.float32)
    W = np.asarray(inputs["in_proj_w"], np.float32)
    convw = np.asarray(inputs["conv_w"], np.float32)
    convb = np.asarray(inputs["conv_b"], np.float32)
    dtb = np.asarray(inputs["dt_bias"], np.float32)
    A = -np.exp(np.asarray(inputs["A_log"], np.float32).astype(np.float64)).astype(np.float32)
    D = np.asarray(inputs["D"], np.float32)
    rmsw = np.asarray(inputs["rms_w"], np.float32)
    Wout = np.asarray(inputs["out_proj_w"], np.float32)
    lng = np.asarray(inputs["ln_g"], np.float32)
    lnb = np.asarray(inputs["ln_b"], np.float32)
    w1 = np.asarray(inputs["w1"], np.float32)
    b1 = np.asarray(inputs["b1"], np.float32)
    w2 = np.asarray(inputs["w2"], np.float32)
    b2 = np.asarray(inputs["b2"], np.float32)

    w1eff = w1[:, :DM] + w1[:, DM:]
    wout_f = Wout * rmsw[None, :]
    w1g = w1eff * lng[None, :]
    b1f = (b1.astype(np.float64) + w1eff.astype(np.float64) @ lnb.astype(np.float64)).astype(np.float32)
    common = {
        "wz": np.ascontiguousarray(W[0:DI].T.reshape(4, 128, DI)),
        "wxbc": np.ascontiguousarray(W[DI:DI + CD].T.reshape(4, 128, CD)),
        "wdt": np.ascontiguousarray(W[DI + CD:].T.reshape(4, 128, NH)),
        "wout": np.ascontiguousarray(wout_f.T.reshape(8, 128, DM)).astype(bf),
        "w1": np.ascontiguousarray(w1g.T.reshape(4, 128, DI)).astype(bf),
        "w2": np.ascontiguousarray(w2.T.reshape(8, 128, DM)).astype(bf),
        "convw": np.ascontiguousarray(convw.reshape(9, 128, 4).transpose(1, 0, 2)),
        "convb": np.ascontiguousarray(convb.reshape(9, 128, 1).transpose(1, 0, 2)),
        "dtb": np.ascontiguousarray(dtb.reshape(NH, 1)),
        "Ah": np.ascontiguousarray(A.reshape(NH, 1)),
        "Drep": np.ascontiguousarray(np.repeat(D, HD).reshape(1, DI)).astype(bf),
        "b1r": np.ascontiguousarray(b1f.reshape(1, 8, 128)),
        "b2r": np.ascontiguousarray(b2.reshape(1, 4, 128)),
        "cbr": np.ascontiguousarray(convb[:1024].reshape(1, 8, 128)),
        "triu": np.ascontiguousarray(
            np.triu(np.ones((128, 128), np.float32)) * np.float32(np.exp(8.0))),
        "indr": _make_indr(),
        "indm": _make_indm(),
        "ltc2": _make_ltc2(),
    }

    x_rev = x[:, ::-1, :]
    in_maps = []
    for core in range(8):
        b, half = core // 2, core % 2
        if half == 0:
            seg = np.vstack([np.zeros((HALO, DM), np.float32), x_rev[b, :SEG]])
        else:
            seg = x_rev[b, SEG - HALO:2 * SEG]
        m = dict(common)
        m["xT"] = np.ascontiguousarray(seg.T.reshape(4, 128, HALO + SEG))
        in_maps.append(m)
    return in_maps


_RT = None


def _prepare_runtime(nc, in_maps):
    """Persistent fast-dispatch path: jit the shard_map'd bass_exec once,
    park the (per-core identical) weight arrays on the 8 devices, and build
    an on-device zeros allocator for the donated output buffers."""
    import jax
    import jax.numpy as jnp
    from jax.sharding import Mesh, PartitionSpec, NamedSharding
    from jax.experimental.shard_map import shard_map
    from concourse import bass2jax, mybir
    bass2jax.install_neuronx_cc_hook()

    n_cores = len(in_maps)
    partition_name = (nc.partition_id_tensor.name
                      if nc.partition_id_tensor else None)
    in_names, out_names, out_avals = [], [], []
    for alloc in nc.m.functions[0].allocations:
        if not isinstance(alloc, mybir.MemoryLocationSet):
            continue
        name = alloc.memorylocations[0].name
        if alloc.kind == "ExternalInput":
            if name != partition_name:
                in_names.append(name)
        elif alloc.kind == "ExternalOutput":
            out_names.append(name)
            out_avals.append(jax.core.ShapedArray(tuple(alloc.tensor_shape),
                                                  mybir.dt.np(alloc.dtype)))
    n_params = len(in_names)
    donate = tuple(range(n_params, n_params + len(out_names)))
    bind_names = list(in_names) + list(out_names)
    if partition_name is not None:
        bind_names.append(partition_name)

    def _body(*args):
        operands = list(args)
        if partition_name is not None:
            operands.append(bass2jax.partition_id_tensor())
        outs = bass2jax._bass_exec_p.bind(
            *operands,
            out_avals=tuple(out_avals),
            in_names=tuple(bind_names),
            out_names=tuple(out_names),
            lowering_input_output_aliases=(),
            sim_require_finite=True,
            sim_require_nnan=True,
            nc=nc,
        )
        return tuple(outs)

    devices = jax.devices()[:n_cores]
    mesh = Mesh(np.asarray(devices), ("core",))
    spec = PartitionSpec("core")
    sharding = NamedSharding(mesh, spec)
    in_specs = (spec,) * (n_params + len(out_names))
    out_specs = (spec,) * len(out_names)
    fn = jax.jit(shard_map(_body, mesh=mesh, in_specs=in_specs,
                           out_specs=out_specs, check_rep=False),
                 donate_argnums=donate, keep_unused=True)

    dev_in = {}
    for name in in_names:
        arrs = [np.asarray(m[name]) for m in in_maps]
        cat = np.concatenate(arrs, axis=0)
        dev_in[name] = jax.device_put(cat, sharding)

    zero_shapes = [(n_cores * a.shape[0], *a.shape[1:]) for a in out_avals]

    def _zeros():
        return [jnp.zeros(s, a.dtype) for s, a in zip(zero_shapes, out_avals)]

    zeros_fn = jax.jit(_zeros, out_shardings=[sharding] * len(out_avals))
    return dict(fn=fn, zeros_fn=zeros_fn, in_names=in_names,
                out_names=out_names, out_avals=out_avals, dev_in=dev_in,
                sharding=sharding, n_cores=n_cores)


def _run(rt, x_cats):
    import jax
    args = []
    for name in rt["in_names"]:
        if name in x_cats:
            args.append(jax.device_put(x_cats[name], rt["sharding"]))
        else:
            args.append(rt["dev_in"][name])
    scratch = rt.pop("_scratch", None)
    if scratch is None:
        scratch = rt["zeros_fn"]()
    outs = rt["fn"](*args, *scratch)
    rt["_scratch"] = outs
    return outs


def _prep_x(inputs):
    x = np.asarray(inputs["x"], np.float32)
    x_rev = x[:, ::-1, :]
    segs = []
    for core in range(8):
        b, half = core // 2, core % 2
        if half == 0:
            seg = np.vstack([np.zeros((HALO, DM), np.float32), x_rev[b, :SEG]])
        else:
            seg = x_rev[b, SEG - HALO:2 * SEG]
        segs.append(seg.T.reshape(4, 128, HALO + SEG))
    return np.ascontiguousarray(np.concatenate(segs, axis=0))


_W_KEYS = ("in_proj_w", "conv_w", "conv_b", "dt_bias", "A_log", "D", "rms_w",
           "out_proj_w", "ln_g", "ln_b", "w1", "b1", "w2", "b2")


def kernel(**inputs):
    global _RT, _BUILT
    import jax
    flags = []
    lnb_ = np.asarray(inputs["ln_b"], np.float64)
    w1eff_ = (np.asarray(inputs["w1"], np.float64)[:, :DM]
              + np.asarray(inputs["w1"], np.float64)[:, DM:])
    b1f_ = np.asarray(inputs["b1"], np.float64) + w1eff_ @ lnb_
    if (not np.any(np.asarray(inputs["conv_b"]))
            and not np.any(b1f_) and not np.any(np.asarray(inputs["b2"]))):
        flags.append("zb")
    if np.all(np.asarray(inputs["D"], np.float64) == 1.0):
        flags.append("oneD")
    flags = tuple(flags)
    if _BUILT is not None and _FLAGS != flags:
        _BUILT = None
        _RT = None
    nc = _build(flags)
    fp = tuple(float(np.asarray(inputs[k], np.float64).sum()) for k in _W_KEYS)
    if _RT is None:
        in_maps = _host_prep(inputs)
        _RT = _prepare_runtime(nc, in_maps)
        _RT["_const_key"] = fp
    elif fp != _RT["_const_key"]:
        in_maps = _host_prep(inputs)
        for name in _RT["in_names"]:
            if name == "xT":
                continue
            cat = np.concatenate([np.asarray(m[name]) for m in in_maps], axis=0)
            _RT["dev_in"][name] = jax.device_put(cat, _RT["sharding"])
        _RT["_const_key"] = fp
    xcat = _prep_x(inputs)
    outs = _run(_RT, {"xT": xcat})
    o = np.asarray(outs[_RT["out_names"].index("outT")])
    o = o.reshape(8, 4, 128, SEG)
    x = np.asarray(inputs["x"])
    out_rev = np.zeros((B, L, DM), np.float32)
    for core in range(8):
        b, half = core // 2, core % 2
        out_rev[b, half * SEG:(half + 1) * SEG] = o[core].reshape(DM, SEG).T
    out = np.ascontiguousarray(out_rev[:, ::-1, :])
    return out.astype(x.dtype)



# revision 70
# speedup vs baseline: 1.0840x; 1.0840x over previous
"""Trainium2 Bass kernel for nn_BiMambaBlock — software-pipelined redesign.

Sharding: batch x sequence-halves -> 8 cores, 128-token halo, host
reverses time.  Key device-program structure:

  - Decay masks exp(S_t - S_s + lndt) are built ON THE PE as a rank-66
    matmul (indicator rows + S split into 16*K + R so the f32r operand
    rounding stays exact) and evicted through e^8*sigmoid(x-8), which
    equals exp(min(x, 8)) to 1e-3.  No DRAM mask broadcast, no DVE
    subtract/min.
  - The D*xh skip term rides the intra-chunk PSUM accumulation via
    identity matmuls; the state update (B(*)dtw masks + diag(elast)
    matmuls) accumulates into one packed [128,512] PSUM tile so the
    cross-tile state chain is just matmuls + one eviction.
  - The gated RMSNorm's per-token scale cancels in the downstream
    LayerNorm (rms_w folded into wout); only sum(yg^2) is kept and fed
    to the LN as a per-token eps correction (1e-5*ms/DI).
  - The tail (out_proj/LN/MLP) of tile i-1 is emitted as a queue of
    pieces interleaved into tile i's SSD stalls, so the PE stream
    always has dense matmul work; PSUM: pa/pc 2-buf heads, acc 2-buf
    1-bank accumulators.
  - Evictions are split across ACT/DVE/GPSIMD by time-window load;
    conv runs as PE diag-matmuls with block 8 (B/C) first.
"""

import numpy as np

# ---- dims ----
DM = 512          # d_model
DST = 64          # d_state
DI = 1024         # d_inner
NH = 16           # heads
HD = 64           # head dim
CD = 1152         # conv dim = DI + 2*DST
B, L = 4, 4096
EPS = 1e-5
HALO, SEG = 128, 2048
TOK = 256         # tokens per pipeline tile
NT = 1 + SEG // TOK   # 9 (halo half-tile + 8 full)
Q = 128           # SSD chunk
NCQ = TOK // Q    # chunks per tile

_BUILT = None
_FLAGS = None
DEBUG = False
REPEAT = 1


def _patch_concourse(tile_mod, bass_mod):
    """This container's walrus accepts a single sync-wait per instruction.
    Split extra waits onto NoOp / extra Drain instructions."""
    from concourse.vector_clock import ScopedClock
    import json

    def _drain_and_barrier(self, tick_clock, wait_clock):
        nc = self.nc
        drain_inst = nc.sync.drain()
        wait_clock.add_sem_waits(drain_inst.ins,
                                 ScopedClock({None: tick_clock.global_clock}))
        si = drain_inst.ins.sync_info
        waits = list(si.on_wait) if (si is not None and si.on_wait) else []
        if len(waits) > 1:
            si.on_wait = waits[:1]
            name2h = {h.name: h for h in self.sems.allocated().values()}
            for w in waits[1:]:
                d2 = nc.sync.drain()
                d2.wait_op(name2h[w.ant_name], w.wait_value, "sem-ge")
        nc.all_engine_barrier()
        popped = nc._tile_sem_poison_stack.pop()
        assert popped is self._sem_poison
        nc.clear_and_free_semaphores(list(self.sems.allocated().values()))
        nc.all_engine_barrier()

    tile_mod.TileContext._drain_and_barrier = _drain_and_barrier

    def _split_waits(m):
        n = 0
        for f in m.get("functions", []):
            for bb in f.get("blocks", []):
                out = []
                for ins in bb.get("instructions", []):
                    si = ins.get("sync_info")
                    waits = (si or {}).get("on_wait") or []
                    if len(waits) > 1:
                        for i, w in enumerate(waits[:-1]):
                            out.append({
                                "debug": ins.get("debug", 0),
                                "engine": ins["engine"],
                                "ins": [], "outs": [],
                                "name": f"{ins['name']}-ws{i}",
                                "opcode": "NoOp",
                                "sync_info": {"on_update": [], "on_wait": [w]},
                            })
                        si["on_wait"] = waits[-1:]
                        n += 1
                    out.append(ins)
                bb["instructions"] = out
        return n

    if not getattr(bass_mod.Bass, "_wait_split_patched", False):
        orig = bass_mod.Bass.to_json_bytes

        def to_json_bytes(self):
            raw = orig(self)
            m = json.loads(raw)
            if _split_waits(m):
                raw = json.dumps(m).encode()
            return raw

        bass_mod.Bass.to_json_bytes = to_json_bytes
        bass_mod.Bass._wait_split_patched = True


def _build(flags=("zb", "oneD")):
    global _BUILT, _FLAGS
    if _BUILT is not None and _FLAGS == flags:
        return _BUILT
    _FLAGS = flags
    zb = "zb" in flags      # conv_b, b1(+ln_b folded), b2 all zero
    oneD = "oneD" in flags  # D == 1
    import concourse.bass as bass
    import concourse.tile as tile
    from concourse import mybir
    from concourse.masks import make_identity
    from contextlib import ExitStack

    _patch_concourse(tile, bass)

    f32 = mybir.dt.float32
    f32r = mybir.dt.float32r
    bf16 = mybir.dt.bfloat16
    AL = mybir.AluOpType
    AF = mybir.ActivationFunctionType

    nc = bass.Bass()

    # ---- DRAM I/O (per-core) ----
    xT = nc.dram_tensor("xT", (4, 128, HALO + SEG), f32r, kind="ExternalInput")
    wz = nc.dram_tensor("wz", (4, 128, DI), f32r, kind="ExternalInput")
    wxbc = nc.dram_tensor("wxbc", (4, 128, CD), f32r, kind="ExternalInput")
    wdt = nc.dram_tensor("wdt", (4, 128, NH), f32r, kind="ExternalInput")
    wout = nc.dram_tensor("wout", (8, 128, DM), bf16, kind="ExternalInput")
    w1 = nc.dram_tensor("w1", (4, 128, DI), bf16, kind="ExternalInput")
    w2 = nc.dram_tensor("w2", (8, 128, DM), bf16, kind="ExternalInput")
    convw = nc.dram_tensor("convw", (128, 9, 4), f32, kind="ExternalInput")
    convb = nc.dram_tensor("convb", (128, 9, 1), f32, kind="ExternalInput")
    dtb = nc.dram_tensor("dtb", (NH, 1), f32, kind="ExternalInput")
    Ah = nc.dram_tensor("Ah", (NH, 1), f32, kind="ExternalInput")
    Drep = nc.dram_tensor("Drep", (1, DI), bf16, kind="ExternalInput")
    b1r = nc.dram_tensor("b1r", (1, 8, 128), f32, kind="ExternalInput")
    b2r = nc.dram_tensor("b2r", (1, 4, 128), f32, kind="ExternalInput")
    cbr = nc.dram_tensor("cbr", (1, 8, 128), f32, kind="ExternalInput")
    triu = nc.dram_tensor("triu", (128, 128), f32, kind="ExternalInput")
    indr = nc.dram_tensor("indr", (66, NH * Q + NH), f32r,
                          kind="ExternalInput")
    indm = nc.dram_tensor("indm", (66, 2 * NH), f32r, kind="ExternalInput")
    ltc2 = nc.dram_tensor("ltc2", (2, 128), f32r, kind="ExternalInput")
    outT = nc.dram_tensor("outT", (4, 128, SEG), f32, kind="ExternalOutput")
    dbg = {}
    if DEBUG:
        for nm, shp, dt_ in [
                ("d_ub", (128, NH, Q), bf16), ("d_dtw", (128, NH), bf16),
                ("d_es", (128, NH), bf16), ("d_cbt", (128, 128), bf16),
                ("d_state", (128, 512), bf16), ("d_ynt", (128, DI), bf16),
                ("d_xh", (128, DI), bf16), ("d_z", (128, DI), bf16),
                ("d_ym", (128, 4, TOK), bf16), ("d_ln", (128, 4, TOK), bf16),
                ("d_sfm", (NH, Q), f32), ("d_lt", (66, Q), f32),
                ("d_rh", (66, NH * Q + NH), f32),
                ("d_conv", (128, 9, TOK), bf16)]:
            dbg[nm] = nc.dram_tensor(nm, shp, dt_, kind="ExternalOutput")

    with tile.TileContext(nc) as tc, ExitStack() as ctx:
        wp = ctx.enter_context(tc.tile_pool(name="wp", bufs=1))
        xp = ctx.enter_context(tc.tile_pool(name="xp", bufs=2))
        cq = ctx.enter_context(tc.tile_pool(name="cq", bufs=2))   # per-tile mids
        sp = ctx.enter_context(tc.tile_pool(name="sp", bufs=2))   # small per-chunk
        mk = ctx.enter_context(tc.tile_pool(name="mk", bufs=2))   # masks
        yp = ctx.enter_context(tc.tile_pool(name="yp", bufs=1))   # big per-chunk
        zp = ctx.enter_context(tc.tile_pool(name="zp", bufs=2))   # xh, z
        st = ctx.enter_context(tc.tile_pool(name="st", bufs=2))   # state & stash
        op = ctx.enter_context(tc.tile_pool(name="op", bufs=2))   # out stage
        dp = ctx.enter_context(tc.tile_pool(name="dp", bufs=2, space="DRAM"))
        # PSUM rings: head (in_proj/conv/dt), chunk (z/transposes/CBt),
        # accumulators (ypsA/ypsB/yn-transposes/pdh, one serial slot),
        # tail (LN stats/out_proj/MLP).  2+2+2+2 banks.
        pa = ctx.enter_context(tc.tile_pool(name="pa", bufs=2, space="PSUM"))
        pc = ctx.enter_context(tc.tile_pool(name="pc", bufs=2, space="PSUM"))
        acc = ctx.enter_context(tc.tile_pool(name="acc", bufs=2, space="PSUM"))
        pt = ctx.enter_context(tc.tile_pool(name="pt", bufs=2, space="PSUM"))

        # ---- load weights / constants ----
        def ld(name, dram, shape, dt=f32):
            t = wp.tile(list(shape), dt, tag=name)
            nc.sync.dma_start(out=t[:], in_=dram[:])
            return t

        t_wz = [ld(f"wz{k}", wz[k], (128, DI), f32r) for k in range(4)]
        t_wxbc = [ld(f"wxbc{k}", wxbc[k], (128, CD), f32r) for k in range(4)]
        t_wdt = [ld(f"wdt{k}", wdt[k], (128, NH), f32r) for k in range(4)]
        t_wout = [ld(f"wout{k}", wout[k], (128, DM), bf16) for k in range(8)]
        t_w1 = [ld(f"w1{k}", w1[k], (128, DI), bf16) for k in range(4)]
        t_w2 = [ld(f"w2{k}", w2[k], (128, DM), bf16) for k in range(8)]
        t_convw = ld("convw", convw, (128, 9, 4))
        t_convb = None if zb else ld("convb", convb, (128, 9, 1))
        t_dtb = ld("dtb", dtb, (NH, 1))
        t_A = ld("Ah", Ah, (NH, 1))
        t_b1r = None if zb else ld("b1r", b1r, (1, 8, 128))
        t_b2r = None if zb else ld("b2r", b2r, (1, 4, 128))
        t_cbr = None if zb else ld("cbr", cbr, (1, 8, 128))
        t_triu = ld("triu", triu, (128, 128))
        t_Dbc = None
        if not oneD:
            t_Dbc = wp.tile([128, DI], bf16, tag="Dbc")
            nc.sync.dma_start(out=t_Dbc[:], in_=Drep[:].to_broadcast((128, DI)))
        ident = wp.tile([128, 128], f32, tag="ident")
        make_identity(nc, ident[:])
        ident_bf = wp.tile([128, 128], bf16, tag="ident_bf")
        nc.vector.tensor_copy(ident_bf[:], ident[:])
        E8 = float(np.exp(8.0))
        ident64_e8 = wp.tile([128, DST], bf16, tag="ident64_e8")
        nc.vector.tensor_scalar(ident64_e8[0:DST, :], ident[0:DST, 0:DST],
                                E8, None, op0=AL.mult)
        nc.vector.tensor_scalar(ident64_e8[DST:128, :], ident[0:DST, 0:DST],
                                E8, None, op0=AL.mult)
        onerow_bf = wp.tile([1, 128], bf16, tag="onerow_bf")
        nc.vector.memset(onerow_bf[:], 1.0)
        # decay-diff matmul rhs: row0 = S-row (dynamic), rows 1..16 = head
        # indicators, cols 2048:2064 = S_last block (ind rows zero there)
        rh_t = []
        for par in range(2):
            t = wp.tile([66, NH * Q + NH], f32r, tag=f"rh{par}",
                        name=f"rh{par}")
            nc.sync.dma_start(out=t[:], in_=indr[:])
            rh_t.append(t)
        rhm = wp.tile([66, 2 * NH], f32r, tag="rhm")
        nc.sync.dma_start(out=rhm[:], in_=indm[:])
        t_ltc2 = wp.tile([2, 128], f32r, tag="ltc2")
        nc.sync.dma_start(out=t_ltc2[:], in_=ltc2[:])
        # conv diag matrices: convd[:, mt, k, :] = diag(convw[:, mt, k]) bf16
        convd = wp.tile([128, 9, 4, 128], bf16, tag="convd")
        for mt in range(9):
            for k in range(4):
                nc.vector.tensor_scalar(convd[:, mt, k, :], ident_bf[:],
                                        t_convw[:, mt, k:k + 1], None,
                                        op0=AL.mult)
        zeros16 = wp.tile([NH, 1], f32, tag="zeros16")
        nc.vector.memset(zeros16[:], 0.0)
        ones16 = wp.tile([NH, 1], f32, tag="ones16")
        nc.vector.memset(ones16[:], 1.0)
        epsc = wp.tile([128, 1], f32, tag="epsc")
        nc.vector.memset(epsc[:], EPS)
        neg8 = wp.tile([128, 1], f32, tag="neg8")
        nc.vector.memset(neg8[:], -8.0)
        z66 = wp.tile([66, 128], f32, tag="z66")
        nc.vector.memset(z66[:], 0.0)
        ones32 = wp.tile([128, 1], f32, tag="ones32")
        nc.vector.memset(ones32[:], 1.0)
        ones_bf = wp.tile([128, 1], bf16, tag="ones_bf")
        nc.vector.tensor_copy(ones_bf[:], ones32[:])
        onerow = None
        if not zb:
            onerow = wp.tile([1, 256], f32, tag="onerow")
            nc.vector.memset(onerow[:], 1.0)

        # persistent state, packed [ (h//8)*64+n , (h%8)*64+d ]
        state = st.tile([128, 512], bf16, tag="state")
        nc.vector.memset(state[:], 0.0)
        stash = st.tile([128, 9, 3], bf16, tag="stash")
        nc.vector.memset(stash[:], 0.0)

        for _rep in range(REPEAT):
            _ = _rep
            tailq = []

            def fill(n=1):
                for _ in range(min(n, len(tailq))):
                    tailq.pop(0)()

            def _emit_tail(it_, t0_, yn_, ssr_):
                tl = {}

                def p_out(mp_):
                    def f():
                        if "ym" not in tl:
                            tl["ym"] = op.tile([128, 4, TOK], bf16,
                                               tag="ym_fm", name="ym_fm")
                        ym_fm = tl["ym"]
                        ps = pt.tile([128, 512], f32, tag="pt", name="ps")
                        for j in range(2):
                            mt = 2 * mp_ + j
                            for k in range(8):
                                nc.tensor.matmul(
                                    ps[:, j * 256:(j + 1) * 256],
                                    t_wout[k][:, mt * 128:(mt + 1) * 128],
                                    yn_[:, k, :], start=(k == 0),
                                    stop=(k == 7))
                        nc.scalar.copy(
                            ym_fm[:, 2 * mp_:2 * mp_ + 2, :]
                                .rearrange("p a b -> p (a b)"), ps[:])
                        if DEBUG and it_ == 1 and mp_ == 1:
                            nc.sync.dma_start(out=dbg["d_ym"][:], in_=ym_fm[:])
                    return f

                def p_lnstats():
                    ym_fm = tl["ym"]
                    sq4 = op.tile([128, 4, TOK], bf16, tag="sq4", name="sq4")
                    nc.vector.tensor_tensor(out=sq4[:], in0=ym_fm[:],
                                            in1=ym_fm[:], op=AL.mult)
                    pmu = pt.tile([1, TOK], f32, tag="pt", name="pmu")
                    for k in range(4):
                        nc.tensor.matmul(pmu[:], ones_bf[:], ym_fm[:, k, :],
                                         start=(k == 0), stop=(k == 3))
                    pv = pt.tile([1, TOK], f32, tag="pt", name="pv")
                    for k in range(4):
                        nc.tensor.matmul(pv[:], ones_bf[:], sq4[:, k, :],
                                         start=(k == 0), stop=(k == 3))
                    murs = sp.tile([1, 2, TOK], bf16, tag="murs", name="murs")
                    nc.scalar.activation(murs[:, 0, :], pmu[:], AF.Copy,
                                         scale=1.0 / DM)
                    ex2_row = sp.tile([1, TOK], f32, tag="ex2_row",
                                      name="ex2_row")
                    nc.scalar.activation(ex2_row[:], pv[:], AF.Copy,
                                         scale=1.0 / DM)
                    var_row = sp.tile([1, TOK], f32, tag="var_row",
                                      name="var_row")
                    nc.vector.tensor_tensor(out=var_row[:], in0=murs[:, 0, :],
                                            in1=murs[:, 0, :], op=AL.mult)
                    nc.vector.tensor_tensor(out=var_row[:], in0=ex2_row[:],
                                            in1=var_row[:], op=AL.subtract)
                    nc.vector.scalar_tensor_tensor(
                        out=var_row[:], in0=ssr_[:], scalar=EPS / DI,
                        in1=var_row[:], op0=AL.mult, op1=AL.add)
                    nc.scalar.activation(var_row[:], var_row[:], AF.Ln,
                                         bias=0.0, scale=1.0)
                    nc.scalar.activation(murs[:, 1, :], var_row[:], AF.Exp,
                                         scale=-0.5)
                    pbc = pc.tile([128, 2, TOK], f32, tag="pc", name="pbc")
                    nc.tensor.matmul(pbc[:].rearrange("p a b -> p (a b)"),
                                     onerow_bf[:],
                                     murs[:].rearrange("p a b -> p (a b)"),
                                     start=True, stop=True)
                    mrbc = sp.tile([128, 2, TOK], bf16, tag="mrbc",
                                   name="mrbc")
                    nc.scalar.copy(mrbc[:].rearrange("p a b -> p (a b)"),
                                   pbc[:].rearrange("p a b -> p (a b)"))
                    ln_fm = op.tile([128, 4, TOK], bf16, tag="sq4",
                                    name="ln_fm")
                    nc.vector.tensor_tensor(
                        out=ln_fm[:], in0=ym_fm[:],
                        in1=mrbc[:, 0:1, :].to_broadcast((128, 4, TOK)),
                        op=AL.subtract)
                    nc.vector.tensor_tensor(
                        out=ln_fm[:], in0=ln_fm[:],
                        in1=mrbc[:, 1:2, :].to_broadcast((128, 4, TOK)),
                        op=AL.mult)
                    if DEBUG and it_ == 1:
                        nc.sync.dma_start(out=dbg["d_ln"][:], in_=ln_fm[:])
                    tl["ln"] = ln_fm

                def p_w1(g):
                    def f():
                        if "h" not in tl:
                            tl["h"] = op.tile([128, 8, TOK], bf16,
                                              tag="h_fm", name="h_fm")
                        h_fm = tl["h"]
                        ln_fm = tl["ln"]
                        for mp_ in (2 * g, 2 * g + 1):
                            ps = pt.tile([128, 512], f32, tag="pt", name="ps")
                            for j in range(2):
                                mt = 2 * mp_ + j
                                reg = ps[:, j * 256:(j + 1) * 256]
                                for k in range(4):
                                    nc.tensor.matmul(
                                        reg,
                                        t_w1[k][:, mt * 128:(mt + 1) * 128],
                                        ln_fm[:, k, :], start=(k == 0),
                                        stop=(zb and k == 3))
                                if not zb:
                                    nc.tensor.matmul(reg, t_b1r[0:1, mt, :],
                                                     onerow[:], start=False,
                                                     stop=True)
                            nc.scalar.activation(
                                h_fm[:, 2 * mp_:2 * mp_ + 2, :]
                                    .rearrange("p a b -> p (a b)"),
                                ps[:], AF.Silu)
                    return f

                def p_w2(g):
                    def f():
                        h_fm = tl["h"]
                        for mt in (2 * g, 2 * g + 1):
                            ps = pt.tile([128, TOK], f32, tag="pt", name="ps")
                            for k in range(8):
                                nc.tensor.matmul(
                                    ps[:],
                                    t_w2[k][:, mt * 128:(mt + 1) * 128],
                                    h_fm[:, k, :], start=(k == 0),
                                    stop=(zb and k == 7))
                            if not zb:
                                nc.tensor.matmul(ps[:], t_b2r[0:1, mt, :],
                                                 onerow[:], start=False,
                                                 stop=True)
                            ot = op.tile([128, TOK], f32, tag="ot", name="ot")
                            nc.scalar.copy(ot[:], ps[:])
                            nc.sync.dma_start(
                                out=outT[mt][:, t0_ - HALO:t0_ - HALO + TOK],
                                in_=ot[:])
                    return f

                tailq.extend([p_out(0), p_out(1), p_lnstats,
                              p_w1(0), p_w1(1), p_w2(0), p_w2(1)])

            for it in range(NT):
                tt = HALO if it == 0 else TOK
                nct = 1 if it == 0 else NCQ
                t0 = 0 if it == 0 else HALO + (it - 1) * TOK
                # ---- x tile (feature-major), one DMA ----
                x_fm = xp.tile([128, 4, tt], f32r, tag="x_fm",
                               padded_shape=[128, 4, TOK])
                for k in range(4):
                    nc.sync.dma_start(out=x_fm[:, k, :],
                                      in_=xT[k][:, t0:t0 + tt])

                # ---- dt path first: its scan + DMA latency feeds the
                # decay matmuls, so start it before the big GEMMs ----
                psd = pa.tile([NH, tt], f32, tag="pa",
                              padded_shape=[NH, TOK])
                for k in range(4):
                    nc.tensor.matmul(psd[:], t_wdt[k][:], x_fm[:, k, :],
                                     start=(k == 0), stop=(k == 3))
                dt_fm = cq.tile([NH, tt], f32, tag="dt_fm",
                                padded_shape=[NH, TOK])
                nc.scalar.activation(dt_fm[:], psd[:], AF.Exp, bias=t_dtb[:],
                                     scale=1.0)
                nc.scalar.activation(dt_fm[:], dt_fm[:], AF.Ln, bias=ones16[:],
                                     scale=1.0)
                dtA_fm = cq.tile([NH, tt], f32, tag="dtA_fm",
                                 padded_shape=[NH, TOK])
                nc.vector.tensor_scalar_mul(dtA_fm[:], dt_fm[:], t_A[:])
                lndt_fm = cq.tile([NH, tt], f32, tag="lndt_fm",
                                  padded_shape=[NH, TOK])
                nc.scalar.activation(lndt_fm[:], dt_fm[:], AF.Ln)
                s_fm = []
                for c in range(nct):
                    sf = sp.tile([NH, Q], f32, tag=f"s_fm{c}")
                    nc.vector.tensor_tensor_scan(
                        sf[:], dtA_fm[:, c * 128:(c + 1) * 128],
                        zeros16[:].to_broadcast((NH, Q)),
                        0.0, op0=AL.add, op1=AL.add)
                    s_fm.append(sf)
                # decay operand prep: f32r rounds operands to ~12 mantissa
                # bits, so the raw cumsum (|S| up to ~6k) loses the ~1e-3
                # absolute precision the decay mask needs.  Split
                # S = 16*K + R (K exactly representable, R in (-16,16)) and
                # carry K and R through separate contraction rows:
                #   lt rows 0..15  = lndt - R   (paired with +1 ind)
                #   lt rows 32..47 = K          (paired with -16 ind)
                #   lt row 64 = 1 (pairs R-row), row 65 = 16 (K-row)
                lt_t = []
                for c in range(nct):
                    Kb = sp.tile([NH, Q], bf16, tag=f"Kb{c}")
                    nc.vector.tensor_scalar(Kb[:], s_fm[c][:], 1.0 / 16.0,
                                            None, op0=AL.mult)
                    Km = sp.tile([NH, Q], f32, tag=f"Km{c}")
                    nc.vector.tensor_copy(Km[:], Kb[:])
                    Rm = sp.tile([NH, Q], f32, tag=f"Rm{c}")
                    nc.vector.scalar_tensor_tensor(
                        out=Rm[:], in0=Km[:], scalar=-16.0, in1=s_fm[c][:],
                        op0=AL.mult, op1=AL.add)
                    lt = sp.tile([66, Q], f32r, tag=f"lt{c}")
                    nc.vector.tensor_copy(lt[:], z66[:])
                    nc.vector.tensor_copy(lt[64:66, :], t_ltc2[:])
                    nc.vector.tensor_tensor(
                        out=lt[0:NH, :], in0=lndt_fm[:, c * Q:(c + 1) * Q],
                        in1=Rm[:], op=AL.subtract)
                    nc.vector.tensor_copy(lt[32:32 + NH, :], Km[:])
                    lt_t.append(lt)
                    if it > 0:
                        rh = rh_t[c]
                        nc.sync.dma_start(out=rh[64:65, :NH * Q],
                                          in_=Rm[:].bitcast(f32r))
                        nc.sync.dma_start(out=rh[64:65, NH * Q:],
                                          in_=Rm[:, Q - 1:Q].bitcast(f32r))
                        nc.sync.dma_start(out=rh[65:66, :NH * Q],
                                          in_=Km[:].bitcast(f32r))
                        nc.sync.dma_start(out=rh[65:66, NH * Q:],
                                          in_=Km[:, Q - 1:Q].bitcast(f32r))
                    else:
                        nc.sync.dma_start(out=rhm[64:65, 0:NH],
                                          in_=Rm[:, Q - 1:Q].bitcast(f32r))
                        nc.sync.dma_start(out=rhm[64:65, NH:2 * NH],
                                          in_=Rm[:, Q - 1:Q].bitcast(f32r))
                        nc.sync.dma_start(out=rhm[65:66, 0:NH],
                                          in_=Km[:, Q - 1:Q].bitcast(f32r))
                        nc.sync.dma_start(out=rhm[65:66, NH:2 * NH],
                                          in_=Km[:, Q - 1:Q].bitcast(f32r))

                # ---- in_proj: paired blocks -> PSUM -> bf16 (conv halo) ----
                xbc_ext = cq.tile([128, 9, 3 + tt], bf16, tag="xbc_ext",
                                  padded_shape=[128, 9, 3 + TOK])
                nc.vector.tensor_copy(xbc_ext[:, :, 0:3], stash[:])
                for mts in ([8], [0, 1], [2, 3], [4, 5], [6, 7]):
                    w = tt * len(mts)
                    ps = pa.tile([128, w], f32, tag="pa",
                                 padded_shape=[128, 512])
                    for j, mt in enumerate(mts):
                        for k in range(4):
                            nc.tensor.matmul(
                                ps[:, j * tt:(j + 1) * tt],
                                t_wxbc[k][:, mt * 128:(mt + 1) * 128],
                                x_fm[:, k, :], start=(k == 0), stop=(k == 3))
                    nc.vector.tensor_copy(
                        xbc_ext[:, mts[0]:mts[0] + len(mts), 3:3 + tt], ps[:])
                stash_new = st.tile([128, 9, 3], bf16, tag="stash")
                nc.vector.tensor_copy(stash_new[:], xbc_ext[:, :, tt:tt + 3])
                stash = stash_new

                # ---- conv: paired diag matmuls + SiLU evict (block 8 first
                # so the B/C features are ready early) ----
                conv_all = cq.tile([128, 9, tt], bf16, tag="conv_all",
                                   padded_shape=[128, 9, TOK])
                for mts in ([8], [0, 1], [2, 3], [4, 5], [6, 7]):
                    w = tt * len(mts)
                    pcv = pa.tile([128, w], f32, tag="pa",
                                  padded_shape=[128, 512])
                    for j, mt in enumerate(mts):
                        reg = pcv[:, j * tt:(j + 1) * tt]
                        for k in range(4):
                            nc.tensor.matmul(reg, convd[:, mt, k, :],
                                             xbc_ext[:, mt, k:k + tt],
                                             start=(k == 0),
                                             stop=((zb or mt == 8) and k == 3))
                        if mt != 8 and not zb:
                            nc.tensor.matmul(reg, t_cbr[0:1, mt, :],
                                             onerow[:, 0:tt],
                                             start=False, stop=True)
                    if mts == [8]:
                        nc.scalar.activation(conv_all[:, 8, :], pcv[:], AF.Silu,
                                             bias=(0.0 if zb else t_convb[:, 8, :]))
                    elif tt == TOK:
                        nc.scalar.activation(
                            conv_all[:, mts[0]:mts[0] + 2, :]
                                .rearrange("p a b -> p (a b)"),
                            pcv[:], AF.Silu)
                    else:
                        for j, mt in enumerate(mts):
                            nc.scalar.activation(
                                conv_all[:, mt, :],
                                pcv[:, j * tt:(j + 1) * tt], AF.Silu)
                conv_s = [conv_all[:, mt, :] for mt in range(9)]
                # ---- B/C prep (conv block 8 first => early) ----
                C_ts, B_tms = [], []
                for c in range(nct):
                    csl = slice(c * 128, (c + 1) * 128)
                    C_t = sp.tile([128, Q], bf16, tag=f"C_t{c}", name="C_t")
                    nc.sync.dma_start(out=C_t[0:DST, :],
                                      in_=conv_s[8][DST:128, csl])
                    nc.sync.dma_start(out=C_t[DST:128, :],
                                      in_=conv_s[8][DST:128, csl])
                    pbt = pc.tile([128, DST], bf16, tag="pc", name="pbt")
                    nc.tensor.transpose(pbt[:], conv_s[8][0:DST, csl],
                                        ident_bf[0:DST, 0:DST])
                    B_tm = sp.tile([128, DST], bf16, tag=f"B_tm{c}",
                                   name="B_tm")
                    nc.vector.tensor_copy(B_tm[:], pbt[:])
                    C_ts.append(C_t)
                    B_tms.append(B_tm)

                # ---- z (token-major, silu) ----
                silu_z = [None, None]
                if it > 0:
                    for c in range(NCQ):
                        zt = zp.tile([128, DI], bf16, tag=f"silu_z{c}")
                        for h2 in range(2):
                            ps = pc.tile([128, 512], f32, tag="pc")
                            for k in range(4):
                                nc.tensor.matmul(
                                    ps[:], x_fm[:, k, c * 128:(c + 1) * 128],
                                    t_wz[k][:, h2 * 512:(h2 + 1) * 512],
                                    start=(k == 0), stop=(k == 3))
                            nc.scalar.activation(zt[:, h2 * 512:(h2 + 1) * 512],
                                                 ps[:], AF.Silu)
                        silu_z[c] = zt

                # ---- xh token-major (transpose + DVE evict) ----
                xh_tm = [None, None]
                for c in range(nct):
                    csl = slice(c * 128, (c + 1) * 128)
                    xt_ = zp.tile([128, DI], bf16, tag=f"xh_tm{c}")
                    ptx = pc.tile([128, 8, 128], bf16, tag="pc")
                    for mt in range(8):
                        nc.tensor.transpose(ptx[:, mt, :], conv_s[mt][:, csl],
                                            ident_bf[:])
                    nc.vector.tensor_copy(xt_[:], ptx[:].rearrange("p a b -> p (a b)"))
                    xh_tm[c] = xt_

                # ---- decay masks (sigmoid evicts land right before their
                # consumption in the chunk loop) ----
                E_tmb = None
                if it > 0:
                    pstb = pc.tile([128, 2, NH], f32, tag="pc")
                    for c in range(NCQ):
                        nc.tensor.transpose(pstb[:, c, :], s_fm[c][:],
                                            ident[0:NH, 0:NH])
                    E_tmb = sp.tile([128, 2, NH], bf16, tag="E_tmb")
                    nc.scalar.activation(E_tmb[:], pstb[:], AF.Exp)
                ubf = [None, None]
                dtw = [None, None]
                es = [None, None]
                mpds, des = [], []
                for c in range(nct):
                    lt = lt_t[c]
                    if it > 0:
                        rh = rh_t[c]
                        ub = mk.tile([128, NH, Q], bf16, tag="ubf")
                        ubfl = ub[:].rearrange("p h q -> p (h q)")
                        for j in range(4):
                            pd = pc.tile([128, 512], f32, tag="pc")
                            nc.tensor.matmul(pd[:], lt[:],
                                             rh[:, j * 512:(j + 1) * 512],
                                             start=True, stop=True)
                            nc.scalar.activation(
                                ubfl[:, j * 512:(j + 1) * 512], pd[:],
                                AF.Sigmoid, bias=neg8[:], scale=1.0)
                        pe_t = pc.tile([128, NH], f32, tag="pc")
                        nc.tensor.matmul(pe_t[:], lt[:],
                                         rh[:, NH * Q:],
                                         start=True, stop=True)
                        ubf[c] = ub
                    else:
                        pm = pc.tile([128, 2 * NH], f32, tag="pc")
                        nc.tensor.matmul(pm[:], lt[:],
                                         rhm[:],
                                         start=True, stop=True)
                        um = sp.tile([128, 2 * NH], bf16, tag=f"um{c}")
                        nc.scalar.activation(um[:], pm[:], AF.Sigmoid,
                                             bias=neg8[:], scale=1.0)
                        pe_t = None
                    dw = sp.tile([128, NH], bf16, tag=f"dtw{c}")
                    esb = sp.tile([128, NH], bf16, tag=f"es{c}")
                    if it > 0:
                        nc.vector.tensor_scalar(dw[:], ubf[c][:, :, Q - 1],
                                                E8, None, op0=AL.mult)
                        nc.scalar.activation(esb[:], pe_t[:],
                                             AF.Sigmoid, bias=neg8[:],
                                             scale=1.0)
                    else:
                        nc.vector.tensor_scalar(dw[:], um[:, 0:NH],
                                                E8, None, op0=AL.mult)
                        nc.vector.tensor_copy(esb[:], um[:, NH:2 * NH])
                    if DEBUG and it == 1 and c == 0:
                        nc.sync.dma_start(out=dbg["d_ub"][:], in_=ubf[c][:])
                        nc.sync.dma_start(out=dbg["d_dtw"][:], in_=dw[:])
                        nc.sync.dma_start(out=dbg["d_es"][:], in_=esb[:])
                        nc.sync.dma_start(out=dbg["d_sfm"][:], in_=s_fm[c][:])
                        nc.sync.dma_start(out=dbg["d_lt"][:],
                                          in_=lt[:].bitcast(f32))
                        nc.sync.dma_start(out=dbg["d_rh"][:],
                                          in_=rh_t[0][:].bitcast(f32))
                    dtw[c] = dw
                    es[c] = esb
                    mpd = yp.tile([128, NH, DST], bf16, tag=f"mpd{c}",
                                  name="mpd")
                    eng_ = nc.vector if c == 0 else nc.gpsimd
                    eng_.tensor_tensor(
                        out=mpd[:],
                        in0=B_tms[c][:].rearrange("p (o n) -> p o n", o=1)
                            .to_broadcast((128, NH, DST)),
                        in1=dw[:].rearrange("p (h o) -> p h o", o=1)
                            .to_broadcast((128, NH, DST)),
                        op=AL.mult)
                    de = yp.tile([128, NH, DST], bf16, tag=f"de{c}", name="de")
                    nc.gpsimd.tensor_tensor(
                        out=de[:],
                        in0=ident64_e8[:].rearrange("p (o n) -> p o n", o=1)
                            .to_broadcast((128, NH, DST)),
                        in1=esb[:].rearrange("p (h o) -> p h o", o=1)
                            .to_broadcast((128, NH, DST)),
                        op=AL.mult)
                    mpds.append(mpd)
                    des.append(de)

                # ---- per 128-chunk SSD ----
                yn_fm = None
                ssr = None
                if it > 0:
                    yn_fm = op.tile([128, 8, TOK], bf16, tag="yn_fm")
                    ssr = sp.tile([1, TOK], f32, tag="ssr", name="ssr")
                for c in range(nct):
                    csl = slice(c * 128, (c + 1) * 128)
                    B_fm = conv_s[8][0:DST, csl]
                    C_t = C_ts[c]
                    mpd = mpds[c]
                    de = des[c]
                    # ---- state update first (needs only mpd/de + old
                    # state, so state(c+1) is ready as early as possible):
                    #   pdh[n,(h,d)] = sum_s B[s,n]*dtw[s,h]*xh[s,(h,d)]
                    #               + elast_h * state[n,(h,d)]   (packed)
                    pdh = acc.tile([128, 512], f32, tag="acc", name="pdh")
                    for h in range(NH):
                        hb, hr = divmod(h, 8)
                        reg = pdh[hb * 64:(hb + 1) * 64,
                                  hr * HD:(hr + 1) * HD]
                        st_sl = state[hb * 64:(hb + 1) * 64,
                                      hr * HD:(hr + 1) * HD]
                        nc.tensor.matmul(reg, mpd[:, h, :],
                                         xh_tm[c][:, h * HD:(h + 1) * HD],
                                         start=True, stop=False)
                        nc.tensor.matmul(
                            reg, de[hb * 64:(hb + 1) * 64, h, :], st_sl,
                            start=False, stop=True)
                    state_new = st.tile([128, 512], bf16, tag="state")
                    nc.scalar.copy(state_new[:], pdh[:])
                    fill(1)
                    if it > 0:
                        # CBt (shared across heads), upper-tri masked
                        pcb = pc.tile([128, 128], f32, tag="pc")
                        nc.tensor.matmul(pcb[:], B_fm, C_t[0:DST, :],
                                         start=True, stop=True)
                        CBt = sp.tile([128, 128], bf16, tag="CBt")
                        nc.vector.tensor_tensor(out=CBt[:], in0=pcb[:],
                                                in1=t_triu[:], op=AL.mult)
                        if DEBUG and it == 1 and c == 0:
                            nc.sync.dma_start(out=dbg["d_cbt"][:], in_=CBt[:])
                        mth = ubf[c]
                        nc.vector.tensor_tensor(
                            out=mth[:], in0=ubf[c][:],
                            in1=CBt[:].rearrange("p (o q) -> p o q", o=1)
                                .to_broadcast((128, NH, Q)),
                            op=AL.mult)

                        if oneD:
                            Dxh = xh_tm[c]
                        else:
                            Dxh = yp.tile([128, DI], bf16, tag="Dxh")
                            nc.gpsimd.tensor_tensor(out=Dxh[:], in0=xh_tm[c][:],
                                                    in1=t_Dbc[:], op=AL.mult)
                        Yt = yp.tile([128, NH, HD], bf16, tag="Yt",
                                     bufs=2)
                        Ytf = Yt[:].rearrange("p h d -> p (h d)")
                        for hb in range(2):
                            # inter-chunk: C @ state-half, scaled by E
                            yB = acc.tile([128, 512], f32, tag="acc",
                                          name="yB")
                            nc.tensor.matmul(yB[:],
                                             C_t[hb * 64:(hb + 1) * 64, :],
                                             state[hb * 64:(hb + 1) * 64, :],
                                             start=True, stop=True)
                            nc.vector.tensor_tensor(
                                out=Yt[:, hb * 8:(hb + 1) * 8, :],
                                in0=yB[:].rearrange("p (h d) -> p h d", h=8),
                                in1=E_tmb[:, c:c + 1, hb * 8:(hb + 1) * 8]
                                    .rearrange("p o h -> p h o")
                                    .to_broadcast((128, 8, HD)),
                                op=AL.mult)
                            if hb == 0:
                                fill(1)
                            # intra-chunk masked matmuls + D*xh skip term
                            yA = acc.tile([128, 512], f32, tag="acc",
                                          name="yA")
                            for hr in range(8):
                                h = hb * 8 + hr
                                reg = yA[:, hr * HD:(hr + 1) * HD]
                                nc.tensor.matmul(
                                    reg, mth[:, h, :],
                                    xh_tm[c][:, h * HD:(h + 1) * HD],
                                    start=True, stop=False)
                                nc.tensor.matmul(
                                    reg, ident_bf[:],
                                    Dxh[:, h * HD:(h + 1) * HD],
                                    start=False, stop=True)
                            nc.vector.tensor_tensor(
                                out=Ytf[:, hb * 512:(hb + 1) * 512],
                                in0=Ytf[:, hb * 512:(hb + 1) * 512],
                                in1=yA[:], op=AL.add)
                        nc.vector.tensor_tensor(out=Ytf, in0=Ytf,
                                                in1=silu_z[c][:], op=AL.mult)
                    state = state_new
                    if it > 0:
                        if DEBUG and it == 1 and c == 0:
                            nc.sync.dma_start(out=dbg["d_ynt"][:], in_=Ytf)
                        fill(1)
                        # transpose y-gated -> feature-major (batched evict)
                        ptn = pc.tile([128, 8, 128], bf16, tag="pc",
                                      name="ptn")
                        for mt in range(8):
                            nc.tensor.transpose(ptn[:, mt, :],
                                                Ytf[:, mt * 128:(mt + 1) * 128],
                                                ident_bf[:])
                        nc.vector.tensor_copy(yn_fm[:, :, csl], ptn[:])
                        # per-token sum(yg^2): the RMS scale itself cancels
                        # in the downstream LayerNorm, but its eps does not;
                        # ship ms to the LN as a per-token eps correction.
                        # Emitted after the yn evict so it stays off the
                        # chunk's forward chain.
                        sqd = yp.tile([128, DI], bf16, tag="sqd")
                        ss = sp.tile([128, 1], f32, tag=f"ss{c}")
                        nc.vector.scalar_tensor_tensor(
                            out=sqd[:], in0=Ytf, scalar=1.0, in1=Ytf,
                            op0=AL.mult, op1=AL.mult, accum_out=ss[:])
                        pss = pc.tile([1, Q], f32, tag="pc", name="pss")
                        nc.tensor.transpose(pss[:], ss[:], ident[:])
                        nc.scalar.copy(ssr[:, c * Q:(c + 1) * Q], pss[:])

                if it > 0:
                    _emit_tail(it, t0, yn_fm, ssr)
                while len(tailq) > 7:
                    fill(1)
            if _rep == REPEAT - 1:
                while tailq:
                    fill(1)


    _BUILT = nc
    return nc


def _make_indr():
    ind = np.zeros((66, NH * 128 + NH), np.float32)
    for h in range(NH):
        ind[h, h * 128:(h + 1) * 128] = 1.0
        ind[32 + h, h * 128:(h + 1) * 128] = -16.0
    return ind


def _make_indm():
    ind = np.zeros((66, 2 * NH), np.float32)
    for h in range(NH):
        ind[h, h] = 1.0
        ind[32 + h, h] = -16.0
    return ind


def _make_ltc2():
    c = np.empty((2, 128), np.float32)
    c[0] = 1.0
    c[1] = 16.0
    return c


def _host_prep(inputs):
    import ml_dtypes
    bf = ml_dtypes.bfloat16
    x = np.asarray(inputs["x"], np.float32)
    W = np.asarray(inputs["in_proj_w"], np.float32)
    convw = np.asarray(inputs["conv_w"], np.float32)
    convb = np.asarray(inputs["conv_b"], np.float32)
    dtb = np.asarray(inputs["dt_bias"], np.float32)
    A = -np.exp(np.asarray(inputs["A_log"], np.float32).astype(np.float64)).astype(np.float32)
    D = np.asarray(inputs["D"], np.float32)
    rmsw = np.asarray(inputs["rms_w"], np.float32)
    Wout = np.asarray(inputs["out_proj_w"], np.float32)
    lng = np.asarray(inputs["ln_g"], np.float32)
    lnb = np.asarray(inputs["ln_b"], np.float32)
    w1 = np.asarray(inputs["w1"], np.float32)
    b1 = np.asarray(inputs["b1"], np.float32)
    w2 = np.asarray(inputs["w2"], np.float32)
    b2 = np.asarray(inputs["b2"], np.float32)

    w1eff = w1[:, :DM] + w1[:, DM:]
    wout_f = Wout * rmsw[None, :]
    w1g = w1eff * lng[None, :]
    b1f = (b1.astype(np.float64) + w1eff.astype(np.float64) @ lnb.astype(np.float64)).astype(np.float32)
    common = {
        "wz": np.ascontiguousarray(W[0:DI].T.reshape(4, 128, DI)),
        "wxbc": np.ascontiguousarray(W[DI:DI + CD].T.reshape(4, 128, CD)),
        "wdt": np.ascontiguousarray(W[DI + CD:].T.reshape(4, 128, NH)),
        "wout": np.ascontiguousarray(wout_f.T.reshape(8, 128, DM)).astype(bf),
        "w1": np.ascontiguousarray(w1g.T.reshape(4, 128, DI)).astype(bf),
        "w2": np.ascontiguousarray(w2.T.reshape(8, 128, DM)).astype(bf),
        "convw": np.ascontiguousarray(convw.reshape(9, 128, 4).transpose(1, 0, 2)),
        "convb": np.ascontiguousarray(convb.reshape(9, 128, 1).transpose(1, 0, 2)),
        "dtb": np.ascontiguousarray(dtb.reshape(NH, 1)),
        "Ah": np.ascontiguousarray(A.reshape(NH, 1)),
        "Drep": np.ascontiguousarray(np.repeat(D, HD).reshape(1, DI)).astype(bf),
        "b1r": np.ascontiguousarray(b1f.reshape(1, 8, 128)),
        "b2r": np.ascontiguousarray(b2.reshape(1, 4, 128)),
        "cbr": np.ascontiguousarray(convb[:1024].reshape(1, 8, 128)),
        "triu": np.ascontiguousarray(
            np.triu(np.ones((128, 128), np.float32)) * np.float32(np.exp(8.0))),
        "indr": _make_indr(),
        "indm": _make_indm(),
        "ltc2": _make_ltc2(),
    }

    x_rev = x[:, ::-1, :]
    in_maps = []
    for core in range(8):
        b, half = core // 2, core % 2
        if half == 0:
            seg = np.vstack([np.zeros((HALO, DM), np.float32), x_rev[b, :SEG]])
        else:
            seg = x_rev[b, SEG - HALO:2 * SEG]
        m = dict(common)
        m["xT"] = np.ascontiguousarray(seg.T.reshape(4, 128, HALO + SEG))
        in_maps.append(m)
    return in_maps


_RT = None


def _prepare_runtime(nc, in_maps):
    """Persistent fast-dispatch path: jit the shard_map'd bass_exec once,
    park the (per-core identical) weight arrays on the 8 devices, and build
    an on-device zeros allocator for the donated output buffers."""
    import jax
    import jax.numpy as jnp
    from jax.sharding import Mesh, PartitionSpec, NamedSharding
    from jax.experimental.shard_map import shard_map
    from concourse import bass2jax, mybir
    bass2jax.install_neuronx_cc_hook()

    n_cores = len(in_maps)
    partition_name = (nc.partition_id_tensor.name
                      if nc.partition_id_tensor else None)
    in_names, out_names, out_avals = [], [], []
    for alloc in nc.m.functions[0].allocations:
        if not isinstance(alloc, mybir.MemoryLocationSet):
            continue
        name = alloc.memorylocations[0].name
        if alloc.kind == "ExternalInput":
            if name != partition_name:
                in_names.append(name)
        elif alloc.kind == "ExternalOutput":
            out_names.append(name)
            out_avals.append(jax.core.ShapedArray(tuple(alloc.tensor_shape),
                                                  mybir.dt.np(alloc.dtype)))
    n_params = len(in_names)
    donate = tuple(range(n_params, n_params + len(out_names)))
    bind_names = list(in_names) + list(out_names)
    if partition_name is not None:
        bind_names.append(partition_name)

    def _body(*args):
        operands = list(args)
        if partition_name is not None:
            operands.append(bass2jax.partition_id_tensor())
        outs = bass2jax._bass_exec_p.bind(
            *operands,
            out_avals=tuple(out_avals),
            in_names=tuple(bind_names),
            out_names=tuple(out_names),
            lowering_input_output_aliases=(),
            sim_require_finite=True,
            sim_require_nnan=True,
            nc=nc,
        )
        return tuple(outs)

    devices = jax.devices()[:n_cores]
    mesh = Mesh(np.asarray(devices), ("core",))
    spec = PartitionSpec("core")
    sharding = NamedSharding(mesh, spec)
    in_specs = (spec,) * (n_params + len(out_names))
    out_specs = (spec,) * len(out_names)
    fn = jax.jit(shard_map(_body, mesh=mesh, in_specs=in_specs,
                           out_specs=out_specs, check_rep=False),
                 donate_argnums=donate, keep_unused=True)

    dev_in = {}
    for name in in_names:
        arrs = [np.asarray(m[name]) for m in in_maps]
        cat = np.concatenate(arrs, axis=0)
        dev_in[name] = jax.device_put(cat, sharding)

    zero_shapes = [(n_cores * a.shape[0], *a.shape[1:]) for a in out_avals]

    def _zeros():
        return [jnp.zeros(s, a.dtype) for s, a in zip(zero_shapes, out_avals)]

    zeros_fn = jax.jit(_zeros, out_shardings=[sharding] * len(out_avals))
    return dict(fn=fn, zeros_fn=zeros_fn, in_names=in_names,
                out_names=out_names, out_avals=out_avals, dev_in=dev_in,
                sharding=sharding, n_cores=n_cores)


def _run(rt, x_cats):
    import jax
    args = []
    for name in rt["in_names"]:
        if name in x_cats:
            args.append(jax.device_put(x_cats[name], rt["sharding"]))
        else:
            args.append(rt["dev_in"][name])
    scratch = rt.pop("_scratch", None)
    if scratch is None:
        scratch = rt["zeros_fn"]()
    outs = rt["fn"](*args, *scratch)
    rt["_scratch"] = outs
    return outs


def _prep_x(inputs):
    x = np.asarray(inputs["x"], np.float32)
    x_rev = x[:, ::-1, :]
    segs = []
    for core in range(8):
        b, half = core // 2, core % 2
        if half == 0:
            seg = np.vstack([np.zeros((HALO, DM), np.float32), x_rev[b, :SEG]])
        else:
            seg = x_rev[b, SEG - HALO:2 * SEG]
        segs.append(seg.T.reshape(4, 128, HALO + SEG))
    return np.ascontiguousarray(np.concatenate(segs, axis=0))


_W_KEYS = ("in_proj_w", "conv_w", "conv_b", "dt_bias", "A_log", "D", "rms_w",
           "out_proj_w", "ln_g", "ln_b", "w1", "b1", "w2", "b2")


def kernel(**inputs):
    global _RT, _BUILT
    import jax
    flags = []
    lnb_ = np.asarray(inputs["ln_b"], np.float64)
    w1eff_ = (np.asarray(inputs["w1"], np.float64)[:, :DM]
              + np.asarray(inputs["w1"], np.float64)[:, DM:])
    b1f_ = np.asarray(inputs["b1"], np.float64) + w1eff_ @ lnb_
    if (not np.any(np.asarray(inputs["conv_b"]))
            and not np.any(b1f_) and not np.any(np.asarray(inputs["b2"]))):
        flags.append("zb")
    if np.all(np.asarray(inputs["D"], np.float64) == 1.0):
        flags.append("oneD")
    flags = tuple(flags)
    if _BUILT is not None and _FLAGS != flags:
        _BUILT = None
        _RT = None
    nc = _build(flags)
    fp = tuple(float(np.asarray(inputs[k], np.float64).sum()) for k in _W_KEYS)
    if _RT is None:
        in_maps = _host_prep(inputs)
        _RT = _prepare_runtime(nc, in_maps)
        _RT["_const_key"] = fp
    elif fp != _RT["_const_key"]:
        in_maps = _host_prep(inputs)
        for name in _RT["in_names"]:
            if name == "xT":
                continue
            cat = np.concatenate([np.asarray(m[name]) for m in in_maps], axis=0)
            _RT["dev_in"][name] = jax.device_put(cat, _RT["sharding"])
        _RT["_const_key"] = fp
    xcat = _prep_x(inputs)
    outs = _run(_RT, {"xT": xcat})
    o = np.asarray(outs[_RT["out_names"].index("outT")])
    o = o.reshape(8, 4, 128, SEG)
    x = np.asarray(inputs["x"])
    out_rev = np.zeros((B, L, DM), np.float32)
    for core in range(8):
        b, half = core // 2, core % 2
        out_rev[b, half * SEG:(half + 1) * SEG] = o[core].reshape(DM, SEG).T
    out = np.ascontiguousarray(out_rev[:, ::-1, :])
    return out.astype(x.dtype)

